# revision 1
# baseline (speedup 1.0000x reference)
"""Trainium2 kernel for nn_DeformationNetworkGraphConvolutionalLite.

Self-contained: accepts FULL inputs (as in reference.setup_inputs()),
shards across 8 NeuronCores internally, returns the FULL [200000, 3] output.
"""
"""Distributed GraphConv deformation network for Trainium2 (8 NeuronCores).

Design:
  - Vertices sharded 8-way (vertex-parallel). Core c owns a contiguous range.
  - Per-core local dest relabeling (host permutation) packs dests into
    196 tiles of 128 such that each (tile, src-owner) slot block holds at
    most 128 directed edges -> every gather chunk maps 1:1 to a dest tile
    (fully structural SPMD graph, no data-dependent control flow).
  - Per layer: y1 = x@W1.T+b1 computed per tile (PE), written to a DRAM
    bounce, AllGather'd into a 8*VLP-row table; dma_gather (int16 local
    idx per owner-slice) pulls neighbor rows; a one-hot (is_equal vs iota)
    matmul scatter-reduces each 128-edge chunk into the dest tile's PSUM
    accumulator; y0 = x@W0.T and biases accumulate into the same PSUM via
    matmuls; ACT applies ReLU into the x buffer in place.
  - Padding slots gather row 0 of a slice and carry dst_rel=-1 so the
    one-hot kills them.
"""
import numpy as np
import concourse.bass as bass
import concourse.bacc as bacc
import concourse.mybir as mybir
import concourse.tile as tile

F32 = mybir.dt.float32
I16 = mybir.dt.int16


def make_cfg(V=200000, E=600000, IMG=960, H=128, NL=8, NC=8, SG=16):
    P = 128
    VL = V // NC
    TILES = (VL + P - 1) // P
    VLP = TILES * P
    cfg = dict(V=V, E=E, IMG=IMG, H=H, NL=NL, NC=NC, P=P, VL=VL, TILES=TILES,
               VLP=VLP, TBL=NC * VLP, SG=SG,
               NSG=(TILES + SG - 1) // SG,
               NCH=NC * TILES,            # chunks per core per layer
               IMGC=(IMG + P - 1) // P)   # padded K chunks for bottleneck
    return cfg


# ---------------------------------------------------------------- host prep

def assign_tiles(cnt, cfg):
    """Greedy balanced assignment of VL dests into TILES tiles of <=128,
    such that per-(tile, owner) edge counts stay <= 128.
    cnt: [VL, NC] per-dest per-owner in-edge counts.
    Returns row_of [VL] -> padded row index in [0, VLP)."""
    P, TILES, NC = cfg["P"], cfg["TILES"], cfg["NC"]
    VL = cfg["VL"]
    rem = np.full((TILES, NC), P, dtype=np.int64)
    slots = np.full(TILES, P, dtype=np.int64)
    order = np.argsort(-cnt.sum(1), kind="stable")
    tile_of = np.empty(VL, dtype=np.int64)
    pos_of = np.empty(VL, dtype=np.int64)
    fill = np.zeros(TILES, dtype=np.int64)
    for d in order:
        v = cnt[d]
        slack = rem - v[None, :]
        ok = (slots > 0) & (slack.min(axis=1) >= 0)
        if not ok.any():
            raise RuntimeError("tile assignment infeasible")
        score = np.where(ok, slack.min(axis=1) * 1000 + slots, -1)
        t = int(np.argmax(score))
        tile_of[d] = t
        pos_of[d] = fill[t]
        fill[t] += 1
        slots[t] -= 1
        rem[t] -= v
    return tile_of * P + pos_of, tile_of, pos_of


def prep_all(edges, cfg):
    """edges: [E,2] int64 global undirected. Returns per-core prep dicts."""
    NC, VL, P, TILES, SG, NSG = (cfg["NC"], cfg["VL"], cfg["P"], cfg["TILES"],
                                 cfg["SG"], cfg["NSG"])
    i, j = edges[:, 0], edges[:, 1]
    dd = np.concatenate([i, j])
    ss = np.concatenate([j, i])
    owner_d = dd // VL
    cores = []
    # pass 1: per-core relabeling
    for c in range(NC):
        m = owner_d == c
        dst = dd[m] - c * VL
        src = ss[m]
        so = src // VL
        cnt = np.bincount(dst * NC + so, minlength=VL * NC).reshape(VL, NC)
        row_of, tile_of, pos_of = assign_tiles(cnt, cfg)
        cores.append(dict(dst=dst, src=src, so=so, row_of=row_of,
                          tile_of=tile_of, pos_of=pos_of))
    row_of_all = [cores[c]["row_of"] for c in range(NC)]
    # block flat layout: for sg: for o: for tile-in-sg: 128 slots
    block_start = np.zeros((TILES, NC), dtype=np.int64)
    cursor = 0
    chunk_of = np.zeros((TILES, NC), dtype=np.int64)
    ch = 0
    for sg in range(NSG):
        t0, t1 = sg * SG, min((sg + 1) * SG, TILES)
        for o in range(NC):
            for t in range(t0, t1):
                block_start[t, o] = cursor
                chunk_of[t, o] = ch
                cursor += P
                ch += 1
    TOT = cursor
    assert ch == cfg["NCH"] and TOT == cfg["NCH"] * P
    # pass 2: slot arrays
    for c in range(NC):
        d = cores[c]
        t_of = d["tile_of"][d["dst"]]
        key = t_of * NC + d["so"]
        ordk = np.argsort(key, kind="stable")
        ks = key[ordk]
        starts = np.searchsorted(ks, np.arange(TILES * NC))
        pos_in_block = np.arange(len(ks)) - starts[ks]
        assert pos_in_block.max(initial=0) < P
        flat = (block_start[t_of, d["so"]][ordk] + pos_in_block)
        idx_arr = np.zeros(TOT, dtype=np.int16)
        drel = np.full(TOT, -1.0, dtype=np.float32)
        srcs = d["src"][ordk]
        local_rows = np.concatenate([row_of_all[o][None] for o in range(NC)])
        # table-local row of each src within its owner's block
        src_local_row = local_rows[srcs // VL, srcs % VL]
        assert src_local_row.max(initial=0) < cfg["VLP"] <= 32768
        idx_arr[flat] = src_local_row.astype(np.int16)
        drel[flat] = d["pos_of"][d["dst"]][ordk].astype(np.float32)
        d["idx16"] = np.ascontiguousarray(idx_arr.reshape(TOT // 16, 16).T)
        d["dst_rel"] = np.ascontiguousarray(
            drel.reshape(cfg["NCH"], P).T)
    return cores, chunk_of


def pack_weights(inp, cfg):
    """Shared (core-independent) aux weight arrays, already transposed."""
    IMG, H, NL, P, IMGC = cfg["IMG"], cfg["H"], cfg["NL"], cfg["P"], cfg["IMGC"]
    f32 = np.float32
    bnWT = np.zeros((IMGC * P, H), f32)
    bnWT[:IMG] = np.asarray(inp["bn_W"], f32).T          # [IMG->pad, H]
    w0 = np.zeros((NL, H, H), f32)
    w1 = np.zeros((NL, H, H), f32)
    b0 = np.zeros((NL, H), f32)
    b1 = np.zeros((NL, H), f32)
    g0W0 = np.asarray(inp["g0_W0"], f32)                 # [H, H+3]
    g0W1 = np.asarray(inp["g0_W1"], f32)
    w0[0] = g0W0[:, :H].T
    w1[0] = g0W1[:, :H].T
    b0[0] = np.asarray(inp["g0_b0"], f32)
    b1[0] = np.asarray(inp["g0_b1"], f32)
    for l in range(1, NL):
        w0[l] = np.asarray(inp["gW0"], f32)[l - 1].T
        w1[l] = np.asarray(inp["gW1"], f32)[l - 1].T
        b0[l] = np.asarray(inp["gb0"], f32)[l - 1]
        b1[l] = np.asarray(inp["gb1"], f32)[l - 1]
    w0v = np.ascontiguousarray(g0W0[:, H:].T)            # [3, H]
    w1v = np.ascontiguousarray(g0W1[:, H:].T)
    voWT = np.ascontiguousarray(np.asarray(inp["vo_W"], f32).T)  # [H, 3]
    return dict(bnWT=bnWT, bnb=np.asarray(inp["bn_b"], f32)[None, :],
                w0=w0, w1=w1, b0=b0.reshape(1, -1), b1=b1.reshape(1, -1),
                w0v=w0v, w1v=w1v,
                voWT=voWT, vob=np.asarray(inp["vo_b"], f32)[None, :],
                iota=np.tile(np.arange(P, dtype=f32), (P, 1)),
                ident=np.eye(P, dtype=f32))


# ---------------------------------------------------------------- builder

def build_nc(cfg):
    import os
    SKIP_AG = os.environ.get("SKIP_AG") == "1"
    SKIP_GATHER = os.environ.get("SKIP_GATHER") == "1"
    SKIP_AGGMM = os.environ.get("SKIP_AGGMM") == "1"
    SKIP_PHASEA = os.environ.get("SKIP_PHASEA") == "1"
    P, H, NL, NC = cfg["P"], cfg["H"], cfg["NL"], cfg["NC"]
    TILES, VLP, TBL, SG, NSG, NCH = (cfg["TILES"], cfg["VLP"], cfg["TBL"],
                                     cfg["SG"], cfg["NSG"], cfg["NCH"])
    IMGC = cfg["IMGC"]
    BPB = 2048 // (H * 4)   # [P,H] f32 regions per 2KB-per-partition PSUM bank
    TOT = NCH * P
    nc = bacc.Bacc(None, target_bir_lowering=False, debug=False)
    dp = lambda n, s: nc.declare_dram_parameter(n, s, F32, isOutput=False)
    img_T = dp("img_T", [IMGC * P, VLP])
    verts = dp("verts", [VLP, 3])
    idx16 = nc.declare_dram_parameter("idx16", [16, TOT // 16], I16, isOutput=False)
    dst_rel = dp("dst_rel", [P, NCH])
    bnWT = dp("bnWT", [IMGC * P, H])
    bnb = dp("bnb", [1, H])
    w0 = dp("w0", [NL, H, H])
    w1 = dp("w1", [NL, H, H])
    b0 = dp("b0", [1, NL * H])
    b1 = dp("b1", [1, NL * H])
    w0v = dp("w0v", [3, H])
    w1v = dp("w1v", [3, H])
    voWT = dp("voWT", [H, 3])
    vob = dp("vob", [1, 3])
    iota_in = dp("iota", [P, P])
    ident_in = dp("ident", [P, P])
    delta = nc.declare_dram_parameter("delta", [VLP, 3], F32, isOutput=True)

    y1b = nc.dram_tensor("y1b", [VLP, H], F32)
    y1full = nc.dram_tensor("y1full", [TBL, H], F32, addr_space="Shared")

    RELU = mybir.ActivationFunctionType.Relu
    EQ = mybir.AluOpType.is_equal

    with tile.TileContext(nc) as tc:
        with tc.tile_pool(name="const", bufs=1) as cpool, \
             tc.tile_pool(name="work", bufs=2) as wpool, \
             tc.tile_pool(name="psum", bufs=1, space="PSUM") as pp, \
             tc.tile_pool(name="psmall", bufs=4, space="PSUM") as ps:

            # ---- resident constants
            x_sb = cpool.tile([P, TILES * H], F32, tag="x")
            verts_sb = cpool.tile([P, TILES * 3], F32, tag="verts")
            nc.sync.dma_start(out=verts_sb[:].rearrange("p (t f) -> p t f", f=3),
                              in_=verts.ap().rearrange("(t p) f -> p t f", p=P))
            idx_t = cpool.tile([P, TOT // 16], I16, tag="idx")
            for cc in range(8):
                nc.sync.dma_start(out=idx_t[cc * 16:(cc + 1) * 16], in_=idx16[:, :])
            drel_t = cpool.tile([P, NCH], F32, tag="drel")
            nc.sync.dma_start(out=drel_t[:], in_=dst_rel[:, :])
            iota_t = cpool.tile([P, P], F32, tag="iota")
            nc.sync.dma_start(out=iota_t[:], in_=iota_in[:, :])
            ident_t = cpool.tile([P, P], F32, tag="ident")
            nc.sync.dma_start(out=ident_t[:], in_=ident_in[:, :])
            ones_t = cpool.tile([1, P], F32, tag="ones")
            nc.vector.memset(ones_t[:], 1.0)
            bnWT_t = cpool.tile([P, IMGC * H], F32, tag="bnWT")
            nc.sync.dma_start(out=bnWT_t[:].rearrange("p (k h) -> p k h", h=H),
                              in_=bnWT.ap().rearrange("(k p) h -> p k h", p=P))
            bnb_t = cpool.tile([1, H], F32, tag="bnb")
            nc.sync.dma_start(out=bnb_t[:], in_=bnb[:, :])
            w0_t = cpool.tile([P, NL * H], F32, tag="w0")
            nc.sync.dma_start(out=w0_t[:].rearrange("p (l h) -> p l h", h=H),
                              in_=w0.ap().rearrange("l p h -> p l h"))
            w1_t = cpool.tile([P, NL * H], F32, tag="w1")
            nc.sync.dma_start(out=w1_t[:].rearrange("p (l h) -> p l h", h=H),
                              in_=w1.ap().rearrange("l p h -> p l h"))
            b0_t = cpool.tile([1, NL * H], F32, tag="b0")
            nc.sync.dma_start(out=b0_t[:], in_=b0[:, :])
            b1_t = cpool.tile([1, NL * H], F32, tag="b1")
            nc.sync.dma_start(out=b1_t[:], in_=b1[:, :])
            w0v_t = cpool.tile([3, H], F32, tag="w0v")
            nc.sync.dma_start(out=w0v_t[:], in_=w0v[:, :])
            w1v_t = cpool.tile([3, H], F32, tag="w1v")
            nc.sync.dma_start(out=w1v_t[:], in_=w1v[:, :])
            voWT_t = cpool.tile([P, 3], F32, tag="voWT")
            nc.sync.dma_start(out=voWT_t[:], in_=voWT[:, :])
            vob_t = cpool.tile([1, 3], F32, tag="vob")
            nc.sync.dma_start(out=vob_t[:], in_=vob[:, :])

            def sg_tiles(sg):
                t0 = sg * SG
                return t0, min(t0 + SG, TILES) - t0

            # ---- stage 0: x = relu(img @ bnW.T + bnb), built per SG block
            for sg in range(NSG):
                t0, ntb = sg_tiles(sg)
                psum = pp.tile([P, SG * H], F32, tag="agg")
                HSG = (SG + 1) // 2
                for kc in range(IMGC):
                    for hb in range(2):
                        h0 = hb * HSG
                        hn = min(HSG, ntb - h0)
                        if hn <= 0:
                            continue
                        imgbuf = wpool.tile([P, HSG * H], F32, tag="img")
                        nc.sync.dma_start(
                            out=imgbuf[:, :hn * H],
                            in_=img_T[kc * P:(kc + 1) * P,
                                      (t0 + h0) * P:(t0 + h0 + hn) * P])
                        for hi in range(hn):
                            ti = h0 + hi
                            nc.tensor.matmul(
                                out=psum[:, ti * H:(ti + 1) * H],
                                lhsT=imgbuf[:, hi * H:(hi + 1) * H],
                                rhs=bnWT_t[:, kc * H:(kc + 1) * H],
                                start=(kc == 0 and ti % BPB == 0), stop=False)
                for ti in range(ntb):
                    nc.tensor.matmul(out=psum[:, ti * H:(ti + 1) * H],
                                     lhsT=ones_t[:1, :], rhs=bnb_t[:1, :],
                                     start=False,
                                     stop=(ti % BPB == BPB - 1 or ti == ntb - 1))
                for ti in range(ntb):
                    nc.scalar.activation(out=x_sb[:, (t0 + ti) * H:(t0 + ti + 1) * H],
                                         in_=psum[:, ti * H:(ti + 1) * H], func=RELU)

            # ---- layers
            for l in range(NL):
                w0l = w0_t[:, l * H:(l + 1) * H]
                w1l = w1_t[:, l * H:(l + 1) * H]
                b0l = b0_t[:1, l * H:(l + 1) * H]
                b1l = b1_t[:1, l * H:(l + 1) * H]

                def xT_of(t, tag):
                    tp = ps.tile([P, P], F32, tag="ps_small")
                    nc.tensor.transpose(out=tp[:], in_=x_sb[:, t * H:(t + 1) * H],
                                        identity=ident_t[:])
                    xs = wpool.tile([P, P], F32, tag=tag)
                    nc.vector.tensor_copy(out=xs[:], in_=tp[:])
                    return xs

                def vT_of(t, tag):
                    tp = ps.tile([P, P], F32, tag="ps_small")
                    nc.tensor.transpose(out=tp[:3, :], in_=verts_sb[:, t * 3:(t + 1) * 3],
                                        identity=ident_t[:])
                    vs = wpool.tile([3, P], F32, tag=tag)
                    nc.vector.tensor_copy(out=vs[:], in_=tp[:3, :])
                    return vs

                # phase A: y1 all tiles -> y1b
                for tb in range(0, TILES, 4):
                    nstage = min(4, TILES - tb)
                    stage = wpool.tile([P, 4 * H], F32, tag="y1stage")
                    for t in ([] if SKIP_PHASEA else range(tb, tb + nstage)):
                        xs = xT_of(t, "xTa")
                        yp = ps.tile([P, P], F32, tag="ps_small")
                        nc.tensor.matmul(out=yp[:], lhsT=xs[:], rhs=w1l,
                                         start=True, stop=False)
                        if l == 0:
                            vs = vT_of(t, "vTa")
                            nc.tensor.matmul(out=yp[:], lhsT=vs[:3, :], rhs=w1v_t[:3, :],
                                             start=False, stop=False)
                        nc.tensor.matmul(out=yp[:], lhsT=ones_t[:1, :], rhs=b1l,
                                         start=False, stop=True)
                        nc.vector.tensor_copy(
                            out=stage[:, (t - tb) * H:(t - tb + 1) * H], in_=yp[:])
                    nc.sync.dma_start(
                        out=y1b[tb * P:(tb + nstage) * P, :].rearrange(
                            "(a p) f -> p a f", p=P),
                        in_=stage[:, :nstage * H].rearrange("p (a f) -> p a f", f=H))

                # phase B: AllGather
                if not SKIP_AG:
                    nc.gpsimd.collective_compute(
                        "AllGather", mybir.AluOpType.bypass,
                        replica_groups=[list(range(NC))],
                        ins=[y1b[:, :]], outs=[y1full[:, :]])

                # phase C: aggregate + y0 + relu
                for sg in range(NSG):
                    t0, ntb = sg_tiles(sg)
                    agg = pp.tile([P, SG * H], F32, tag="agg")
                    ch0 = chunk_base(cfg, sg)
                    for o in range(NC):
                        gbuf = wpool.tile([P, SG, H], F32, tag="g")
                        if not SKIP_GATHER:
                            ids = idx_t[:, (ch0 + o * ntb) * 8:(ch0 + (o + 1) * ntb) * 8]
                            nc.gpsimd.dma_gather(
                                out_ap=gbuf[:, :ntb, :],
                                in_ap=y1full[o * VLP:(o + 1) * VLP, :],
                                idxs_ap=ids, num_idxs=ntb * P, num_idxs_reg=ntb * P,
                                elem_size=H, single_packet=False)
                        S = wpool.tile([P, SG * H], F32, tag="S")
                        dr = drel_t[:, ch0 + o * ntb: ch0 + (o + 1) * ntb]
                        nc.vector.tensor_tensor(
                            out=S[:, :ntb * H].rearrange("p (c f) -> p c f", f=P),
                            in0=dr[:, :, None].to_broadcast([P, ntb, P]),
                            in1=iota_t[:, None, :].to_broadcast([P, ntb, P]),
                            op=EQ)
                        for ti in range(ntb):
                            if SKIP_AGGMM and not (o == 0 and ti % BPB == 0):
                                continue
                            nc.tensor.matmul(
                                out=agg[:, ti * H:(ti + 1) * H],
                                lhsT=S[:, ti * H:(ti + 1) * H],
                                rhs=gbuf[:, ti, :],
                                start=(o == 0 and ti % BPB == 0), stop=False)
                    for ti in range(ntb):
                        t = t0 + ti
                        xs = xT_of(t, "xTc")
                        nc.tensor.matmul(out=agg[:, ti * H:(ti + 1) * H],
                                         lhsT=xs[:], rhs=w0l, start=False, stop=False)
                        if l == 0:
                            vs = vT_of(t, "vTc")
                            nc.tensor.matmul(out=agg[:, ti * H:(ti + 1) * H],
                                             lhsT=vs[:3, :], rhs=w0v_t[:3, :],
                                             start=False, stop=False)
                        nc.tensor.matmul(out=agg[:, ti * H:(ti + 1) * H],
                                         lhsT=ones_t[:1, :], rhs=b0l,
                                         start=False,
                                         stop=(ti % BPB == BPB - 1 or ti == ntb - 1))
                    for ti in range(ntb):
                        t = t0 + ti
                        nc.scalar.activation(out=x_sb[:, t * H:(t + 1) * H],
                                             in_=agg[:, ti * H:(ti + 1) * H],
                                             func=RELU)

            # ---- final projection
            outstage = cpool.tile([P, TILES * 3], F32, tag="outstage")
            for t in range(TILES):
                tp = ps.tile([P, P], F32, tag="ps_small")
                nc.tensor.transpose(out=tp[:], in_=x_sb[:, t * H:(t + 1) * H],
                                    identity=ident_t[:])
                xs = wpool.tile([P, P], F32, tag="xTf")
                nc.vector.tensor_copy(out=xs[:], in_=tp[:])
                op = ps.tile([P, P], F32, tag="ps_small")
                nc.tensor.matmul(out=op[:, :3], lhsT=xs[:], rhs=voWT_t[:, :3],
                                 start=True, stop=False)
                nc.tensor.matmul(out=op[:, :3], lhsT=ones_t[:1, :], rhs=vob_t[:1, :3],
                                 start=False, stop=True)
                nc.vector.tensor_copy(out=outstage[:, t * 3:(t + 1) * 3],
                                      in_=op[:, :3])
            nc.sync.dma_start(
                out=delta.ap().rearrange("(t p) f -> p t f", p=P),
                in_=outstage[:].rearrange("p (t f) -> p t f", f=3))

    nc.finalize()
    return nc


def chunk_base(cfg, sg):
    """Number of 128-slot chunks before super-group sg (all prior sgs full)."""
    return cfg["NC"] * sg * cfg["SG"]


# ------------------------------ runner ------------------------------
import time
import numpy as np
import jax
from jax.sharding import Mesh, PartitionSpec
from jax.experimental.shard_map import shard_map
import concourse.mybir as mybir
from concourse import bass2jax
from concourse.bass2jax import _bass_exec_p, partition_id_tensor, install_neuronx_cc_hook


class SpmdRunner:
    def __init__(self, nc, n_cores=8):
        install_neuronx_cc_hook()
        self.nc = nc
        self.n_cores = n_cores
        partition_name = nc.partition_id_tensor.name if nc.partition_id_tensor else None
        in_names, out_names, out_avals, zero_outs = [], [], [], []
        for alloc in nc.m.functions[0].allocations:
            if not isinstance(alloc, mybir.MemoryLocationSet):
                continue
            name = alloc.memorylocations[0].name
            if alloc.kind == "ExternalInput":
                if name != partition_name:
                    in_names.append(name)
            elif alloc.kind == "ExternalOutput":
                out_names.append(name)
                shape = tuple(alloc.tensor_shape)
                dtype = mybir.dt.np(alloc.dtype)
                out_avals.append(jax.core.ShapedArray(shape, dtype))
                zero_outs.append(np.zeros(shape, dtype))
        self.n_params = len(in_names)
        self.out_names = list(out_names)
        self.out_avals = out_avals
        self.zero_outs = zero_outs
        all_in = in_names + out_names
        if partition_name is not None:
            all_in.append(partition_name)
        self.in_names = all_in
        n_outs = len(out_avals)
        donate = tuple(range(self.n_params, self.n_params + n_outs))

        def _body(*args):
            operands = list(args)
            if partition_name is not None:
                operands.append(partition_id_tensor())
            return tuple(_bass_exec_p.bind(
                *operands,
                out_avals=tuple(out_avals),
                in_names=tuple(self.in_names),
                out_names=tuple(out_names),
                lowering_input_output_aliases=(),
                sim_require_finite=True,
                sim_require_nnan=True,
                nc=nc,
            ))

        devices = jax.devices()[:n_cores]
        mesh = Mesh(np.asarray(devices), ("core",))
        in_specs = (PartitionSpec("core"),) * (self.n_params + n_outs)
        out_specs = (PartitionSpec("core"),) * n_outs
        self.jitted = jax.jit(
            shard_map(_body, mesh=mesh, in_specs=in_specs, out_specs=out_specs,
                      check_rep=False),
            donate_argnums=donate,
            keep_unused=True,
        )

    def _concat_inputs(self, in_maps):
        if self.nc.dbg_addr is not None:
            z = np.zeros((1, 2), np.uint32)
            in_maps = [{**m, self.nc.dbg_addr.name: z} for m in in_maps]
        per_core = [[np.asarray(m[name]) for name in self.in_names[:self.n_params]]
                    for m in in_maps]
        concat_in = [np.concatenate([per_core[c][i] for c in range(self.n_cores)], axis=0)
                     for i in range(self.n_params)]
        concat_zeros = [np.zeros((self.n_cores * z.shape[0], *z.shape[1:]), z.dtype)
                        for z in self.zero_outs]
        return concat_in, concat_zeros

    def run(self, in_maps, iters=0):
        """Returns (results_per_core, best_seconds_per_iter or None)."""
        concat_in, concat_zeros = self._concat_inputs(in_maps)
        # device_put once so timing excludes H2D
        concat_in = [jax.device_put(a) for a in concat_in]
        out_arrs = self.jitted(*concat_in, *[jax.device_put(z) for z in concat_zeros])
        jax.block_until_ready(out_arrs)
        best = None
        for _ in range(iters):
            zs = [jax.device_put(z) for z in concat_zeros]
            jax.block_until_ready(zs)
            t0 = time.perf_counter()
            out_arrs2 = self.jitted(*concat_in, *zs)
            jax.block_until_ready(out_arrs2)
            dt = time.perf_counter() - t0
            best = dt if best is None else min(best, dt)
        results = [
            {name: np.asarray(out_arrs[i]).reshape(self.n_cores, *self.out_avals[i].shape)[c]
             for i, name in enumerate(self.out_names)}
            for c in range(self.n_cores)
        ]
        return results, best


# ------------------------------ host side ------------------------------
"""Host-side orchestration: prep inputs, run the SPMD kernel, assemble output."""
import numpy as np


def make_in_maps(inputs, cfg):
    NC, VL, VLP, IMG, IMGC, P = (cfg["NC"], cfg["VL"], cfg["VLP"], cfg["IMG"],
                                 cfg["IMGC"], cfg["P"])
    edges = np.asarray(inputs["edges"]).astype(np.int64)
    img = np.asarray(inputs["img_feats"], np.float32)
    verts = np.asarray(inputs["verts"], np.float32)
    cores, _ = prep_all(edges, cfg)
    shared = pack_weights(inputs, cfg)
    in_maps = []
    for c in range(NC):
        d = cores[c]
        row_of = d["row_of"]
        img_T = np.zeros((IMGC * P, VLP), np.float32)
        img_T[:IMG, row_of] = img[c * VL:(c + 1) * VL].T
        vpad = np.zeros((VLP, 3), np.float32)
        vpad[row_of] = verts[c * VL:(c + 1) * VL]
        m = dict(shared)
        m.update(img_T=img_T, verts=vpad, idx16=d["idx16"], dst_rel=d["dst_rel"])
        in_maps.append(m)
    return in_maps, cores


def assemble(results, cores, cfg):
    NC, VL, V = cfg["NC"], cfg["VL"], cfg["V"]
    out = np.empty((V, 3), np.float32)
    for c in range(NC):
        out[c * VL:(c + 1) * VL] = results[c]["delta"][cores[c]["row_of"]]
    return out


_CACHE = {}


def _get_runner(cfg):
    key = (cfg["V"], cfg["NL"], cfg["SG"])
    if key not in _CACHE:
        nc = build_nc(cfg)
        _CACHE[key] = SpmdRunner(nc)
    return _CACHE[key]


def kernel(**inputs):
    cfg = make_cfg()
    in_maps, cores = make_in_maps(inputs, cfg)
    r = _get_runner(cfg)
    results, _ = r.run(in_maps, iters=0)
    return assemble(results, cores, cfg)



# revision 2
# speedup vs baseline: 6.5056x; 6.5056x over previous
"""Trainium2 kernel for nn_DeformationNetworkGraphConvolutionalLite.

Self-contained: accepts FULL inputs (as in reference.setup_inputs()),
shards across 8 NeuronCores internally, returns the FULL [200000, 3] output.

Distributed GraphConv deformation network (8 NeuronCores), fp16 fast path.

Design:
  - Vertices sharded 8-way (vertex-parallel). Core c owns a contiguous range.
  - Activations live in SBUF feature-transposed: xT [128=h, VLP=v] fp16.
    All dense matmuls then run at moving-free-dim 512 with no transposes:
      y1 rows  : out[v,h'] = sum_h xT[h,v-tile] * w1[h,h']   (lhsT=xT tile)
      y0T cols : out[h',v] = sum_h w0[h,h'] * xT[h,v]        (rhs =xT cols)
    b0 + ReLU fuse into the Activation-engine PSUM->SBUF copy (per-partition
    bias); b1 is added via a 1-row ones matmul (pre-scatter, as in pytorch3d).
  - Per-core local dest relabeling (host) packs dests into 196 tiles of 128
    such that each (tile, src-owner) block holds at most 128 directed edges.
  - Per layer: y1 (fp16) -> DRAM bounce; AllGather into an 8*VLP-row fp16
    table; dma_gather (int16 idx per owner slice) pulls neighbor rows in
    [slot, h] layout; matmul with a one-hot [slot, dpos] (is_equal vs iota)
    scatter-reduces each 128-edge chunk into the dest tile's PSUM column
    block, accumulating on top of y0T.
  - Padding slots gather row 0 and carry dst_rel=-1 so the one-hot kills
    them. Everything 2-byte (fp16): half the HBM/collective/gather traffic
    and 1 PE cycle/row instead of 4.
"""
import numpy as np
import concourse.bass as bass
import concourse.bacc as bacc
import concourse.mybir as mybir
import concourse.tile as tile

F32 = mybir.dt.float32
F16 = mybir.dt.float16
I16 = mybir.dt.int16


def make_cfg(V=200000, E=600000, IMG=960, H=128, NL=8, NC=8, SG=16):
    P = 128
    VL = V // NC
    TILES = (VL + P - 1) // P
    VLP = TILES * P
    cfg = dict(V=V, E=E, IMG=IMG, H=H, NL=NL, NC=NC, P=P, VL=VL, TILES=TILES,
               VLP=VLP, TBL=NC * VLP, SG=SG,
               NSG=(TILES + SG - 1) // SG,
               NCH=NC * TILES,            # chunks per core per layer
               IMGC=(IMG + P - 1) // P,   # padded K chunks for bottleneck
               GP=4)                      # tiles per PSUM bank group
    return cfg


# ---------------------------------------------------------------- host prep

def assign_tiles(cnt, cfg):
    """Greedy balanced assignment of VL dests into TILES tiles of <=128,
    such that per-(tile, owner) edge counts stay <= 128.
    cnt: [VL, NC] per-dest per-owner in-edge counts.
    Returns row_of [VL] -> padded row index in [0, VLP)."""
    P, TILES, NC = cfg["P"], cfg["TILES"], cfg["NC"]
    VL = cfg["VL"]
    rem = np.full((TILES, NC), P, dtype=np.int64)
    slots = np.full(TILES, P, dtype=np.int64)
    order = np.argsort(-cnt.sum(1), kind="stable")
    tile_of = np.empty(VL, dtype=np.int64)
    pos_of = np.empty(VL, dtype=np.int64)
    fill = np.zeros(TILES, dtype=np.int64)
    for d in order:
        v = cnt[d]
        slack = rem - v[None, :]
        ok = (slots > 0) & (slack.min(axis=1) >= 0)
        if not ok.any():
            raise RuntimeError("tile assignment infeasible")
        score = np.where(ok, slack.min(axis=1) * 1000 + slots, -1)
        t = int(np.argmax(score))
        tile_of[d] = t
        pos_of[d] = fill[t]
        fill[t] += 1
        slots[t] -= 1
        rem[t] -= v
    return tile_of * P + pos_of, tile_of, pos_of


def prep_all(edges, cfg):
    """edges: [E,2] int64 global undirected. Returns per-core prep dicts."""
    NC, VL, P, TILES, SG, NSG = (cfg["NC"], cfg["VL"], cfg["P"], cfg["TILES"],
                                 cfg["SG"], cfg["NSG"])
    i, j = edges[:, 0], edges[:, 1]
    dd = np.concatenate([i, j])
    ss = np.concatenate([j, i])
    owner_d = dd // VL
    cores = []
    # pass 1: per-core relabeling
    for c in range(NC):
        m = owner_d == c
        dst = dd[m] - c * VL
        src = ss[m]
        so = src // VL
        cnt = np.bincount(dst * NC + so, minlength=VL * NC).reshape(VL, NC)
        row_of, tile_of, pos_of = assign_tiles(cnt, cfg)
        cores.append(dict(dst=dst, src=src, so=so, row_of=row_of,
                          tile_of=tile_of, pos_of=pos_of))
    row_of_all = [cores[c]["row_of"] for c in range(NC)]
    # block flat layout: for sg: for o: for tile-in-sg: 128 slots
    block_start = np.zeros((TILES, NC), dtype=np.int64)
    cursor = 0
    chunk_of = np.zeros((TILES, NC), dtype=np.int64)
    ch = 0
    for sg in range(NSG):
        t0, t1 = sg * SG, min((sg + 1) * SG, TILES)
        for o in range(NC):
            for t in range(t0, t1):
                block_start[t, o] = cursor
                chunk_of[t, o] = ch
                cursor += P
                ch += 1
    TOT = cursor
    assert ch == cfg["NCH"] and TOT == cfg["NCH"] * P
    # pass 2: slot arrays
    for c in range(NC):
        d = cores[c]
        t_of = d["tile_of"][d["dst"]]
        key = t_of * NC + d["so"]
        ordk = np.argsort(key, kind="stable")
        ks = key[ordk]
        starts = np.searchsorted(ks, np.arange(TILES * NC))
        pos_in_block = np.arange(len(ks)) - starts[ks]
        assert pos_in_block.max(initial=0) < P
        flat = (block_start[t_of, d["so"]][ordk] + pos_in_block)
        idx_arr = np.zeros(TOT, dtype=np.int16)
        drel = np.full(TOT, -1.0, dtype=np.float16)
        srcs = d["src"][ordk]
        local_rows = np.concatenate([row_of_all[o][None] for o in range(NC)])
        # table-local row of each src within its owner's block
        src_local_row = local_rows[srcs // VL, srcs % VL]
        assert src_local_row.max(initial=0) < cfg["VLP"] <= 32768
        idx_arr[flat] = src_local_row.astype(np.int16)
        drel[flat] = d["pos_of"][d["dst"]][ordk].astype(np.float16)
        d["idx16"] = np.ascontiguousarray(idx_arr.reshape(TOT // 16, 16).T)
        d["dst_rel"] = np.ascontiguousarray(
            drel.reshape(cfg["NCH"], P).T)
    return cores, chunk_of


def pack_weights(inp, cfg):
    """Shared (core-independent) aux weight arrays, already transposed."""
    IMG, H, NL, P, IMGC = cfg["IMG"], cfg["H"], cfg["NL"], cfg["P"], cfg["IMGC"]
    f16, f32 = np.float16, np.float32
    bnWT = np.zeros((IMGC * P, H), f16)
    bnWT[:IMG] = np.asarray(inp["bn_W"], f32).T.astype(f16)   # [IMG->pad, H]
    w0 = np.zeros((NL, H, H), f16)
    w1 = np.zeros((NL, H, H), f16)
    b0 = np.zeros((H, NL), f32)
    b1 = np.zeros((NL, H), f16)
    g0W0 = np.asarray(inp["g0_W0"], f32)                 # [H, H+3]
    g0W1 = np.asarray(inp["g0_W1"], f32)
    w0[0] = g0W0[:, :H].T.astype(f16)
    w1[0] = g0W1[:, :H].T.astype(f16)
    b0[:, 0] = np.asarray(inp["g0_b0"], f32)
    b1[0] = np.asarray(inp["g0_b1"], f16)
    for l in range(1, NL):
        w0[l] = np.asarray(inp["gW0"], f32)[l - 1].T.astype(f16)
        w1[l] = np.asarray(inp["gW1"], f32)[l - 1].T.astype(f16)
        b0[:, l] = np.asarray(inp["gb0"], f32)[l - 1]
        b1[l] = np.asarray(inp["gb1"], f32)[l - 1].astype(f16)
    w0v = np.ascontiguousarray(g0W0[:, H:].T.astype(f16))    # [3, H]
    w1v = np.ascontiguousarray(g0W1[:, H:].T.astype(f16))
    voWT = np.ascontiguousarray(np.asarray(inp["vo_W"], f32).T.astype(f16))
    return dict(bnWT=bnWT,
                bnb=np.asarray(inp["bn_b"], f32).reshape(H, 1),
                w0=w0, w1=w1, b0col=b0, b1=b1.reshape(1, -1),
                w0v=w0v, w1v=w1v,
                voWT=voWT, vob=np.asarray(inp["vo_b"], f16)[None, :],
                iota=np.tile(np.arange(P, dtype=f16), (P, 1)))


# ---------------------------------------------------------------- builder

def build_nc(cfg):
    P, H, NL, NC = cfg["P"], cfg["H"], cfg["NL"], cfg["NC"]
    TILES, VLP, TBL, SG, NSG, NCH = (cfg["TILES"], cfg["VLP"], cfg["TBL"],
                                     cfg["SG"], cfg["NSG"], cfg["NCH"])
    IMGC, GP = cfg["IMGC"], cfg["GP"]
    NG = TILES // GP            # 512-col groups (49)
    GW = GP * P                 # group width in verts (512)
    TOT = NCH * P
    assert TILES % GP == 0
    nc = bacc.Bacc(None, target_bir_lowering=False, debug=False)
    dp16 = lambda n, s: nc.declare_dram_parameter(n, s, F16, isOutput=False)
    img_T = dp16("img_T", [IMGC * P, VLP])
    vertsT = dp16("vertsT", [3, VLP])
    idx16 = nc.declare_dram_parameter("idx16", [16, TOT // 16], I16, isOutput=False)
    dst_rel = dp16("dst_rel", [P, NCH])
    bnWT = dp16("bnWT", [IMGC * P, H])
    bnb = nc.declare_dram_parameter("bnb", [H, 1], F32, isOutput=False)
    w0 = dp16("w0", [NL, H, H])
    w1 = dp16("w1", [NL, H, H])
    b0col = nc.declare_dram_parameter("b0col", [H, NL], F32, isOutput=False)
    b1 = dp16("b1", [1, NL * H])
    w0v = dp16("w0v", [3, H])
    w1v = dp16("w1v", [3, H])
    voWT = dp16("voWT", [H, 3])
    vob = dp16("vob", [1, 3])
    iota_in = dp16("iota", [P, P])
    delta = nc.declare_dram_parameter("delta", [VLP, 3], F32, isOutput=True)

    y1b = nc.dram_tensor("y1b", [VLP, H], F16)
    y1full = nc.dram_tensor("y1full", [TBL, H], F16, addr_space="Shared")

    RELU = mybir.ActivationFunctionType.Relu
    COPY = mybir.ActivationFunctionType.Copy
    EQ = mybir.AluOpType.is_equal

    with tile.TileContext(nc) as tc:
        with tc.tile_pool(name="const", bufs=1) as cpool, \
             tc.tile_pool(name="work", bufs=2) as wpool, \
             tc.tile_pool(name="gpool", bufs=4) as gpool, \
             tc.tile_pool(name="spool", bufs=3) as spool, \
             tc.tile_pool(name="psum", bufs=6, space="PSUM") as pp:

            # ---- resident constants
            xT = cpool.tile([P, VLP], F16, tag="xT")
            idx_t = cpool.tile([P, TOT // 16], I16, tag="idx")
            for cc in range(8):
                nc.sync.dma_start(out=idx_t[cc * 16:(cc + 1) * 16], in_=idx16[:, :])
            drel_t = cpool.tile([P, NCH], F16, tag="drel")
            nc.sync.dma_start(out=drel_t[:], in_=dst_rel[:, :])
            iota_t = cpool.tile([P, P], F16, tag="iota")
            nc.sync.dma_start(out=iota_t[:], in_=iota_in[:, :])
            ones_t = cpool.tile([1, P], F16, tag="ones")
            nc.vector.memset(ones_t[:], 1.0)
            bnWT_t = cpool.tile([P, IMGC * H], F16, tag="bnWT")
            nc.sync.dma_start(out=bnWT_t[:].rearrange("p (k h) -> p k h", h=H),
                              in_=bnWT.ap().rearrange("(k p) h -> p k h", p=P))
            bnb_t = cpool.tile([P, 1], F32, tag="bnb")
            nc.sync.dma_start(out=bnb_t[:], in_=bnb[:, :])
            w0_t = cpool.tile([P, NL * H], F16, tag="w0")
            nc.sync.dma_start(out=w0_t[:].rearrange("p (l h) -> p l h", h=H),
                              in_=w0.ap().rearrange("l p h -> p l h"))
            w1_t = cpool.tile([P, NL * H], F16, tag="w1")
            nc.sync.dma_start(out=w1_t[:].rearrange("p (l h) -> p l h", h=H),
                              in_=w1.ap().rearrange("l p h -> p l h"))
            b0_t = cpool.tile([P, NL], F32, tag="b0")
            nc.sync.dma_start(out=b0_t[:], in_=b0col[:, :])
            b1_t = cpool.tile([1, NL * H], F16, tag="b1")
            nc.sync.dma_start(out=b1_t[:], in_=b1[:, :])
            w0v_t = cpool.tile([3, H], F16, tag="w0v")
            nc.sync.dma_start(out=w0v_t[:], in_=w0v[:, :])
            w1v_t = cpool.tile([3, H], F16, tag="w1v")
            nc.sync.dma_start(out=w1v_t[:], in_=w1v[:, :])
            voWT_t = cpool.tile([P, 3], F16, tag="voWT")
            nc.sync.dma_start(out=voWT_t[:], in_=voWT[:, :])
            vob_t = cpool.tile([1, 3], F16, tag="vob")
            nc.sync.dma_start(out=vob_t[:], in_=vob[:, :])
            outstage = cpool.tile([P, TILES * 3], F32, tag="outstage")

            # ---- stage 0: xT = relu(bnW @ imgT + bnb), 512 verts per group
            for g in range(NG):
                ps = pp.tile([P, GW], F32, tag="bank")
                for kc in range(IMGC):
                    imgbuf = wpool.tile([P, GW], F16, tag="img")
                    nc.sync.dma_start(
                        out=imgbuf[:],
                        in_=img_T[kc * P:(kc + 1) * P, g * GW:(g + 1) * GW])
                    nc.tensor.matmul(out=ps[:], lhsT=bnWT_t[:, kc * H:(kc + 1) * H],
                                     rhs=imgbuf[:],
                                     start=(kc == 0), stop=(kc == IMGC - 1))
                nc.scalar.activation(out=xT[:, g * GW:(g + 1) * GW], in_=ps[:],
                                     func=RELU, bias=bnb_t[:, :1])

            # ---- layers
            for l in range(NL):
                w0l = w0_t[:, l * H:(l + 1) * H]
                w1l = w1_t[:, l * H:(l + 1) * H]
                b1l = b1_t[:1, l * H:(l + 1) * H]
                b0l = b0_t[:, l:l + 1]

                # phase A: y1 rows (pre-scatter bias included) -> y1b
                for g in range(NG):
                    ps = pp.tile([P, GW], F32, tag="bank")
                    if l == 0:
                        vbufA = wpool.tile([3, GW], F16, tag="vA")
                        nc.sync.dma_start(out=vbufA[:],
                                          in_=vertsT[:3, g * GW:(g + 1) * GW])
                    for t in range(GP):
                        col = (g * GP + t) * P
                        sl = ps[:, t * H:(t + 1) * H]
                        nc.tensor.matmul(out=sl, lhsT=xT[:, col:col + P], rhs=w1l,
                                         start=(t == 0), stop=False)
                        if l == 0:
                            nc.tensor.matmul(out=sl, lhsT=vbufA[:3, t * P:(t + 1) * P],
                                             rhs=w1v_t[:3, :], start=False, stop=False)
                        nc.tensor.matmul(out=sl, lhsT=ones_t[:1, :], rhs=b1l,
                                         start=False, stop=(t == GP - 1))
                    y1st = wpool.tile([P, GW], F16, tag="y1st")
                    nc.scalar.activation(out=y1st[:], in_=ps[:], func=COPY)
                    nc.sync.dma_start(
                        out=y1b[g * GW:(g + 1) * GW, :].rearrange(
                            "(a p) f -> p a f", p=P),
                        in_=y1st[:].rearrange("p (a f) -> p a f", f=H))

                # phase B: AllGather (fp16 table)
                nc.gpsimd.collective_compute(
                    "AllGather", mybir.AluOpType.bypass,
                    replica_groups=[list(range(NC))],
                    ins=[y1b[:, :]], outs=[y1full[:, :]])

                # phase C: scatter-reduce + y0T + relu(…+b0)
                for sg in range(NSG):
                    t0 = sg * SG
                    ntb = min(SG, TILES - t0)
                    ch0 = NC * sg * SG
                    if l == 0:
                        vbufC = wpool.tile([3, SG * P], F16, tag="vC")
                        nc.sync.dma_start(out=vbufC[:, :ntb * P],
                                          in_=vertsT[:3, t0 * P:(t0 + ntb) * P])
                    nbank = (ntb + GP - 1) // GP
                    banks = []
                    for b in range(nbank):
                        ps = pp.tile([P, GW], F32, tag="bank")
                        cols = (t0 + b * GP) * P
                        nc.tensor.matmul(out=ps[:], lhsT=w0l,
                                         rhs=xT[:, cols:cols + GW],
                                         start=True, stop=False)
                        if l == 0:
                            nc.tensor.matmul(
                                out=ps[:], lhsT=w0v_t[:3, :],
                                rhs=vbufC[:3, b * GW:(b + 1) * GW],
                                start=False, stop=False)
                        banks.append(ps)
                    for o in range(NC):
                        gb = gpool.tile([P, SG, H], F16, tag="g")
                        ids = idx_t[:, (ch0 + o * ntb) * 8:(ch0 + (o + 1) * ntb) * 8]
                        nc.gpsimd.dma_gather(
                            out_ap=gb[:, :ntb, :],
                            in_ap=y1full[o * VLP:(o + 1) * VLP, :],
                            idxs_ap=ids, num_idxs=ntb * P, num_idxs_reg=ntb * P,
                            elem_size=H, single_packet=False)
                        S = spool.tile([P, SG * P], F16, tag="S")
                        dr = drel_t[:, ch0 + o * ntb: ch0 + (o + 1) * ntb]
                        nc.vector.tensor_tensor(
                            out=S[:, :ntb * P].rearrange("p (c f) -> p c f", f=P),
                            in0=dr[:, :, None].to_broadcast([P, ntb, P]),
                            in1=iota_t[:, None, :].to_broadcast([P, ntb, P]),
                            op=EQ)
                        for t in range(ntb):
                            b = t // GP
                            nc.tensor.matmul(
                                out=banks[b][:, (t % GP) * H:(t % GP + 1) * H],
                                lhsT=gb[:, t, :], rhs=S[:, t * P:(t + 1) * P],
                                start=False,
                                stop=(o == NC - 1 and (t % GP == GP - 1
                                                       or t == ntb - 1)))
                    for b in range(nbank):
                        cols = (t0 + b * GP) * P
                        nc.scalar.activation(out=xT[:, cols:cols + GW],
                                             in_=banks[b][:], func=RELU, bias=b0l)

            # ---- final projection: delta rows = x @ voW.T + vo_b
            for g in range(NG):
                ps = pp.tile([P, GW], F32, tag="bank")
                for t in range(GP):
                    col = (g * GP + t) * P
                    sl = ps[:, t * 3:(t + 1) * 3]
                    nc.tensor.matmul(out=sl, lhsT=xT[:, col:col + P],
                                     rhs=voWT_t[:, :3], start=(t == 0), stop=False)
                    nc.tensor.matmul(out=sl, lhsT=ones_t[:1, :], rhs=vob_t[:1, :3],
                                     start=False, stop=(t == GP - 1))
                nc.vector.tensor_copy(out=outstage[:, g * GP * 3:(g + 1) * GP * 3],
                                      in_=ps[:, :GP * 3])
            nc.sync.dma_start(
                out=delta.ap().rearrange("(t p) f -> p t f", p=P),
                in_=outstage[:].rearrange("p (t f) -> p t f", f=3))

    nc.finalize()
    return nc


# ------------------------------ runner ------------------------------
import time
import numpy as np
import jax
from jax.sharding import Mesh, PartitionSpec
from jax.experimental.shard_map import shard_map
import concourse.mybir as mybir
from concourse import bass2jax
from concourse.bass2jax import _bass_exec_p, partition_id_tensor, install_neuronx_cc_hook


class SpmdRunner:
    def __init__(self, nc, n_cores=8):
        install_neuronx_cc_hook()
        self.nc = nc
        self.n_cores = n_cores
        partition_name = nc.partition_id_tensor.name if nc.partition_id_tensor else None
        in_names, out_names, out_avals, zero_outs = [], [], [], []
        for alloc in nc.m.functions[0].allocations:
            if not isinstance(alloc, mybir.MemoryLocationSet):
                continue
            name = alloc.memorylocations[0].name
            if alloc.kind == "ExternalInput":
                if name != partition_name:
                    in_names.append(name)
            elif alloc.kind == "ExternalOutput":
                out_names.append(name)
                shape = tuple(alloc.tensor_shape)
                dtype = mybir.dt.np(alloc.dtype)
                out_avals.append(jax.core.ShapedArray(shape, dtype))
                zero_outs.append(np.zeros(shape, dtype))
        self.n_params = len(in_names)
        self.out_names = list(out_names)
        self.out_avals = out_avals
        self.zero_outs = zero_outs
        all_in = in_names + out_names
        if partition_name is not None:
            all_in.append(partition_name)
        self.in_names = all_in
        n_outs = len(out_avals)
        donate = tuple(range(self.n_params, self.n_params + n_outs))

        def _body(*args):
            operands = list(args)
            if partition_name is not None:
                operands.append(partition_id_tensor())
            return tuple(_bass_exec_p.bind(
                *operands,
                out_avals=tuple(out_avals),
                in_names=tuple(self.in_names),
                out_names=tuple(out_names),
                lowering_input_output_aliases=(),
                sim_require_finite=True,
                sim_require_nnan=True,
                nc=nc,
            ))

        devices = jax.devices()[:n_cores]
        mesh = Mesh(np.asarray(devices), ("core",))
        in_specs = (PartitionSpec("core"),) * (self.n_params + n_outs)
        out_specs = (PartitionSpec("core"),) * n_outs
        self.jitted = jax.jit(
            shard_map(_body, mesh=mesh, in_specs=in_specs, out_specs=out_specs,
                      check_rep=False),
            donate_argnums=donate,
            keep_unused=True,
        )

    def _concat_inputs(self, in_maps):
        if self.nc.dbg_addr is not None:
            z = np.zeros((1, 2), np.uint32)
            in_maps = [{**m, self.nc.dbg_addr.name: z} for m in in_maps]
        per_core = [[np.asarray(m[name]) for name in self.in_names[:self.n_params]]
                    for m in in_maps]
        concat_in = [np.concatenate([per_core[c][i] for c in range(self.n_cores)], axis=0)
                     for i in range(self.n_params)]
        concat_zeros = [np.zeros((self.n_cores * z.shape[0], *z.shape[1:]), z.dtype)
                        for z in self.zero_outs]
        return concat_in, concat_zeros

    def run(self, in_maps, iters=0):
        """Returns (results_per_core, best_seconds_per_iter or None)."""
        concat_in, concat_zeros = self._concat_inputs(in_maps)
        # device_put once so timing excludes H2D
        concat_in = [jax.device_put(a) for a in concat_in]
        out_arrs = self.jitted(*concat_in, *[jax.device_put(z) for z in concat_zeros])
        jax.block_until_ready(out_arrs)
        best = None
        for _ in range(iters):
            zs = [jax.device_put(z) for z in concat_zeros]
            jax.block_until_ready(zs)
            t0 = time.perf_counter()
            out_arrs2 = self.jitted(*concat_in, *zs)
            jax.block_until_ready(out_arrs2)
            dt = time.perf_counter() - t0
            best = dt if best is None else min(best, dt)
        results = [
            {name: np.asarray(out_arrs[i]).reshape(self.n_cores, *self.out_avals[i].shape)[c]
             for i, name in enumerate(self.out_names)}
            for c in range(self.n_cores)
        ]
        return results, best


# ------------------------------ host side ------------------------------
import numpy as np


def make_in_maps(inputs, cfg):
    NC, VL, VLP, IMG, IMGC, P = (cfg["NC"], cfg["VL"], cfg["VLP"], cfg["IMG"],
                                 cfg["IMGC"], cfg["P"])
    edges = np.asarray(inputs["edges"]).astype(np.int64)
    img16 = np.asarray(inputs["img_feats"]).astype(np.float16)
    verts16 = np.asarray(inputs["verts"]).astype(np.float16)
    cores, _ = prep_all(edges, cfg)
    shared = pack_weights(inputs, cfg)
    in_maps = []
    for c in range(NC):
        d = cores[c]
        row_of = d["row_of"]
        img_T = np.zeros((IMGC * P, VLP), np.float16)
        img_T[:IMG, row_of] = img16[c * VL:(c + 1) * VL].T
        vT = np.zeros((3, VLP), np.float16)
        vT[:, row_of] = verts16[c * VL:(c + 1) * VL].T
        m = dict(shared)
        m.update(img_T=img_T, vertsT=vT, idx16=d["idx16"], dst_rel=d["dst_rel"])
        in_maps.append(m)
    return in_maps, cores


def assemble(results, cores, cfg):
    NC, VL, V = cfg["NC"], cfg["VL"], cfg["V"]
    out = np.empty((V, 3), np.float32)
    for c in range(NC):
        out[c * VL:(c + 1) * VL] = results[c]["delta"][cores[c]["row_of"]]
    return out


_CACHE = {}


def _get_runner(cfg):
    key = (cfg["V"], cfg["NL"], cfg["SG"])
    if key not in _CACHE:
        nc = build_nc(cfg)
        _CACHE[key] = SpmdRunner(nc)
    return _CACHE[key]


def kernel(**inputs):
    cfg = make_cfg()
    in_maps, cores = make_in_maps(inputs, cfg)
    r = _get_runner(cfg)
    results, _ = r.run(in_maps, iters=0)
    return assemble(results, cores, cfg)


# revision 10
# speedup vs baseline: 6.8765x; 1.0570x over previous
"""Trainium2 kernel for nn_DeformationNetworkGraphConvolutionalLite.

Self-contained: accepts FULL inputs (as in reference.setup_inputs()),
shards across 8 NeuronCores internally, returns the FULL [200000, 3] output.

Distributed GraphConv deformation network (8 NeuronCores), fp16 fast path.

Design:
  - Vertices sharded 8-way (vertex-parallel). Core c owns a contiguous range.
  - Activations live in SBUF feature-transposed: xT [128=h, VLP=v] fp16.
    All dense matmuls then run at moving-free-dim 512 with no transposes:
      y1 rows  : out[v,h'] = sum_h xT[h,v-tile] * w1[h,h']   (lhsT=xT tile)
      y0T cols : out[h',v] = sum_h w0[h,h'] * xT[h,v]        (rhs =xT cols)
    b0 + ReLU fuse into the Activation-engine PSUM->SBUF copy (per-partition
    bias); b1 is added via a 1-row ones matmul (pre-scatter, as in pytorch3d).
  - Per-core local dest relabeling (host) packs dests into 196 tiles of 128
    such that each (tile, src-owner) block holds at most 128 directed edges.
  - Per layer: y1 (fp16) -> DRAM bounce; AllGather into an 8*VLP-row fp16
    table; dma_gather (int16 idx per owner slice) pulls neighbor rows in
    [slot, h] layout; matmul with a one-hot [slot, dpos] (is_equal vs iota)
    scatter-reduces each 128-edge chunk into the dest tile's PSUM column
    block, accumulating on top of y0T.
  - Padding slots gather row 0 and carry dst_rel=-1 so the one-hot kills
    them. Everything 2-byte (fp16): half the HBM/collective/gather traffic
    and 1 PE cycle/row instead of 4.
"""
import numpy as np
import concourse.bass as bass
import concourse.bacc as bacc
import concourse.mybir as mybir
import concourse.tile as tile

F32 = mybir.dt.float32
F16 = mybir.dt.float16
I16 = mybir.dt.int16


def make_cfg(V=200000, E=600000, IMG=960, H=128, NL=8, NC=8, SG=16):
    P = 128
    VL = V // NC
    TILES = (VL + P - 1) // P
    VLP = TILES * P
    cfg = dict(V=V, E=E, IMG=IMG, H=H, NL=NL, NC=NC, P=P, VL=VL, TILES=TILES,
               VLP=VLP, TBL=NC * VLP, SG=SG,
               NSG=(TILES + SG - 1) // SG,
               NCH=NC * TILES,            # chunks per core per layer
               IMGC=(IMG + P - 1) // P,   # padded K chunks for bottleneck
               GP=4)                      # tiles per PSUM bank group
    return cfg


# ---------------------------------------------------------------- host prep

def assign_tiles(cnt, cfg):
    """Greedy balanced assignment of VL dests into TILES tiles of <=128,
    such that per-(tile, owner) edge counts stay <= 128.
    cnt: [VL, NC] per-dest per-owner in-edge counts.
    Returns row_of [VL] -> padded row index in [0, VLP)."""
    P, TILES, NC = cfg["P"], cfg["TILES"], cfg["NC"]
    VL = cfg["VL"]
    rem = np.full((TILES, NC), P, dtype=np.int64)
    slots = np.full(TILES, P, dtype=np.int64)
    order = np.argsort(-cnt.sum(1), kind="stable")
    tile_of = np.empty(VL, dtype=np.int64)
    pos_of = np.empty(VL, dtype=np.int64)
    fill = np.zeros(TILES, dtype=np.int64)
    for d in order:
        v = cnt[d]
        slack = rem - v[None, :]
        ok = (slots > 0) & (slack.min(axis=1) >= 0)
        if not ok.any():
            raise RuntimeError("tile assignment infeasible")
        score = np.where(ok, slack.min(axis=1) * 1000 + slots, -1)
        t = int(np.argmax(score))
        tile_of[d] = t
        pos_of[d] = fill[t]
        fill[t] += 1
        slots[t] -= 1
        rem[t] -= v
    return tile_of * P + pos_of, tile_of, pos_of


def prep_all(edges, cfg):
    """edges: [E,2] int64 global undirected. Returns per-core prep dicts."""
    NC, VL, P, TILES, SG, NSG = (cfg["NC"], cfg["VL"], cfg["P"], cfg["TILES"],
                                 cfg["SG"], cfg["NSG"])
    i, j = edges[:, 0], edges[:, 1]
    dd = np.concatenate([i, j])
    ss = np.concatenate([j, i])
    owner_d = dd // VL
    cores = []
    # pass 1: per-core relabeling
    for c in range(NC):
        m = owner_d == c
        dst = dd[m] - c * VL
        src = ss[m]
        so = src // VL
        cnt = np.bincount(dst * NC + so, minlength=VL * NC).reshape(VL, NC)
        row_of, tile_of, pos_of = assign_tiles(cnt, cfg)
        cores.append(dict(dst=dst, src=src, so=so, row_of=row_of,
                          tile_of=tile_of, pos_of=pos_of))
    row_of_all = [cores[c]["row_of"] for c in range(NC)]
    # block flat layout: for sg: for o: for tile-in-sg: 128 slots
    block_start = np.zeros((TILES, NC), dtype=np.int64)
    cursor = 0
    chunk_of = np.zeros((TILES, NC), dtype=np.int64)
    ch = 0
    for sg in range(NSG):
        t0, t1 = sg * SG, min((sg + 1) * SG, TILES)
        for o in range(NC):
            for t in range(t0, t1):
                block_start[t, o] = cursor
                chunk_of[t, o] = ch
                cursor += P
                ch += 1
    TOT = cursor
    assert ch == cfg["NCH"] and TOT == cfg["NCH"] * P
    # pass 2: slot arrays
    for c in range(NC):
        d = cores[c]
        t_of = d["tile_of"][d["dst"]]
        key = t_of * NC + d["so"]
        ordk = np.argsort(key, kind="stable")
        ks = key[ordk]
        starts = np.searchsorted(ks, np.arange(TILES * NC))
        pos_in_block = np.arange(len(ks)) - starts[ks]
        assert pos_in_block.max(initial=0) < P
        flat = (block_start[t_of, d["so"]][ordk] + pos_in_block)
        idx_arr = np.zeros(TOT, dtype=np.int16)
        drel = np.full(TOT, -1.0, dtype=np.float16)
        srcs = d["src"][ordk]
        local_rows = np.concatenate([row_of_all[o][None] for o in range(NC)])
        # table-local row of each src within its owner's block
        src_local_row = local_rows[srcs // VL, srcs % VL]
        assert src_local_row.max(initial=0) < cfg["VLP"] <= 32768
        idx_arr[flat] = src_local_row.astype(np.int16)
        drel[flat] = d["pos_of"][d["dst"]][ordk].astype(np.float16)
        d["idx16"] = np.ascontiguousarray(idx_arr.reshape(TOT // 16, 16).T)
        d["dst_rel"] = np.ascontiguousarray(
            drel.reshape(cfg["NCH"], P).T)
    return cores, chunk_of


def pack_weights(inp, cfg):
    """Shared (core-independent) aux weight arrays, already transposed."""
    IMG, H, NL, P, IMGC = cfg["IMG"], cfg["H"], cfg["NL"], cfg["P"], cfg["IMGC"]
    f16, f32 = np.float16, np.float32
    bnWT = np.zeros((IMGC * P, H), f16)
    bnWT[:IMG] = np.asarray(inp["bn_W"], f32).T.astype(f16)   # [IMG->pad, H]
    w0 = np.zeros((NL, H, H), f16)
    w1 = np.zeros((NL, H, H), f16)
    b0 = np.zeros((H, NL), f32)
    b1 = np.zeros((NL, H), f16)
    g0W0 = np.asarray(inp["g0_W0"], f32)                 # [H, H+3]
    g0W1 = np.asarray(inp["g0_W1"], f32)
    w0[0] = g0W0[:, :H].T.astype(f16)
    w1[0] = g0W1[:, :H].T.astype(f16)
    b0[:, 0] = np.asarray(inp["g0_b0"], f32)
    b1[0] = np.asarray(inp["g0_b1"], f16)
    for l in range(1, NL):
        w0[l] = np.asarray(inp["gW0"], f32)[l - 1].T.astype(f16)
        w1[l] = np.asarray(inp["gW1"], f32)[l - 1].T.astype(f16)
        b0[:, l] = np.asarray(inp["gb0"], f32)[l - 1]
        b1[l] = np.asarray(inp["gb1"], f32)[l - 1].astype(f16)
    w0v = np.ascontiguousarray(g0W0[:, H:].T.astype(f16))    # [3, H]
    w1v = np.ascontiguousarray(g0W1[:, H:].T.astype(f16))
    voWT = np.ascontiguousarray(np.asarray(inp["vo_W"], f32).T.astype(f16))
    return dict(bnWT=bnWT,
                bnb=np.asarray(inp["bn_b"], f32).reshape(H, 1),
                w0=w0, w1=w1, b0col=b0, b1=b1.reshape(1, -1),
                w0v=w0v, w1v=w1v,
                voWT=voWT, vob=np.asarray(inp["vo_b"], f16)[None, :],
                iota=np.tile(np.arange(P, dtype=f16), (P, 1)))


# ---------------------------------------------------------------- builder

def build_nc(cfg, skip=frozenset()):
    """skip: subset of {"ag","gather","scatter","phasea"} for ablation
    benches only — results are wrong when non-empty."""
    skip = frozenset(skip)
    P, H, NL, NC = cfg["P"], cfg["H"], cfg["NL"], cfg["NC"]
    TILES, VLP, TBL, SG, NSG, NCH = (cfg["TILES"], cfg["VLP"], cfg["TBL"],
                                     cfg["SG"], cfg["NSG"], cfg["NCH"])
    IMGC, GP = cfg["IMGC"], cfg["GP"]
    NG = TILES // GP            # 512-col groups (49)
    GW = GP * P                 # group width in verts (512)
    TOT = NCH * P
    assert TILES % GP == 0
    nc = bacc.Bacc(None, target_bir_lowering=False, debug=False)
    dp16 = lambda n, s: nc.declare_dram_parameter(n, s, F16, isOutput=False)
    img_T = dp16("img_T", [IMGC * P, VLP])
    vertsT = dp16("vertsT", [3, VLP])
    idx16 = nc.declare_dram_parameter("idx16", [16, TOT // 16], I16, isOutput=False)
    dst_rel = dp16("dst_rel", [P, NCH])
    bnWT = dp16("bnWT", [IMGC * P, H])
    bnb = nc.declare_dram_parameter("bnb", [H, 1], F32, isOutput=False)
    w0 = dp16("w0", [NL, H, H])
    w1 = dp16("w1", [NL, H, H])
    b0col = nc.declare_dram_parameter("b0col", [H, NL], F32, isOutput=False)
    b1 = dp16("b1", [1, NL * H])
    w0v = dp16("w0v", [3, H])
    w1v = dp16("w1v", [3, H])
    voWT = dp16("voWT", [H, 3])
    vob = dp16("vob", [1, 3])
    iota_in = dp16("iota", [P, P])
    delta = nc.declare_dram_parameter("delta", [VLP, 3], F32, isOutput=True)

    y1b = nc.dram_tensor("y1b", [VLP, H], F16)
    y1full = nc.dram_tensor("y1full", [TBL, H], F16, addr_space="Shared")

    RELU = mybir.ActivationFunctionType.Relu
    COPY = mybir.ActivationFunctionType.Copy
    EQ = mybir.AluOpType.is_equal

    with tile.TileContext(nc) as tc:
        with tc.tile_pool(name="const", bufs=1) as cpool, \
             tc.tile_pool(name="work", bufs=2) as wpool, \
             tc.tile_pool(name="gpool", bufs=10) as gpool, \
             tc.tile_pool(name="spool", bufs=4) as spool, \
             tc.tile_pool(name="psum", bufs=6, space="PSUM") as pp:

            # ---- resident constants
            xT = cpool.tile([P, VLP], F16, tag="xT")
            idx_t = cpool.tile([P, TOT // 16], I16, tag="idx")
            for cc in range(8):
                nc.sync.dma_start(out=idx_t[cc * 16:(cc + 1) * 16], in_=idx16[:, :])
            drel_t = cpool.tile([P, NCH], F16, tag="drel")
            nc.sync.dma_start(out=drel_t[:], in_=dst_rel[:, :])
            iota_t = cpool.tile([P, P], F16, tag="iota")
            nc.sync.dma_start(out=iota_t[:], in_=iota_in[:, :])
            ones_t = cpool.tile([1, P], F16, tag="ones")
            nc.vector.memset(ones_t[:], 1.0)
            bnWT_t = cpool.tile([P, IMGC * H], F16, tag="bnWT")
            nc.sync.dma_start(out=bnWT_t[:].rearrange("p (k h) -> p k h", h=H),
                              in_=bnWT.ap().rearrange("(k p) h -> p k h", p=P))
            bnb_t = cpool.tile([P, 1], F32, tag="bnb")
            nc.sync.dma_start(out=bnb_t[:], in_=bnb[:, :])
            w0_t = cpool.tile([P, NL * H], F16, tag="w0")
            nc.sync.dma_start(out=w0_t[:].rearrange("p (l h) -> p l h", h=H),
                              in_=w0.ap().rearrange("l p h -> p l h"))
            w1_t = cpool.tile([P, NL * H], F16, tag="w1")
            nc.sync.dma_start(out=w1_t[:].rearrange("p (l h) -> p l h", h=H),
                              in_=w1.ap().rearrange("l p h -> p l h"))
            b0_t = cpool.tile([P, NL], F32, tag="b0")
            nc.sync.dma_start(out=b0_t[:], in_=b0col[:, :])
            b1_t = cpool.tile([1, NL * H], F16, tag="b1")
            nc.sync.dma_start(out=b1_t[:], in_=b1[:, :])
            w0v_t = cpool.tile([3, H], F16, tag="w0v")
            nc.sync.dma_start(out=w0v_t[:], in_=w0v[:, :])
            w1v_t = cpool.tile([3, H], F16, tag="w1v")
            nc.sync.dma_start(out=w1v_t[:], in_=w1v[:, :])
            voWT_t = cpool.tile([P, 3], F16, tag="voWT")
            nc.sync.dma_start(out=voWT_t[:], in_=voWT[:, :])
            vob_t = cpool.tile([1, 3], F16, tag="vob")
            nc.sync.dma_start(out=vob_t[:], in_=vob[:, :])
            outstage = cpool.tile([P, TILES * 3], F32, tag="outstage")

            def emit_y1_group(l, g):
                """y1 rows for layer l, vert group g (pre-scatter bias
                included) -> y1b. Emitted right after group g's xT update
                so it overlaps the rest of the previous phase."""
                if "phasea" in skip:
                    return
                w1l = w1_t[:, l * H:(l + 1) * H]
                b1l = b1_t[:1, l * H:(l + 1) * H]
                ps = pp.tile([P, GW], F32, tag="bank")
                if l == 0:
                    vbufA = wpool.tile([3, GW], F16, tag="vA")
                    nc.sync.dma_start(out=vbufA[:],
                                      in_=vertsT[:3, g * GW:(g + 1) * GW])
                for t in range(GP):
                    col = (g * GP + t) * P
                    sl = ps[:, t * H:(t + 1) * H]
                    nc.tensor.matmul(out=sl, lhsT=xT[:, col:col + P], rhs=w1l,
                                     start=(t == 0), stop=False)
                    if l == 0:
                        nc.tensor.matmul(out=sl, lhsT=vbufA[:3, t * P:(t + 1) * P],
                                         rhs=w1v_t[:3, :], start=False, stop=False)
                    nc.tensor.matmul(out=sl, lhsT=ones_t[:1, :], rhs=b1l,
                                     start=False, stop=(t == GP - 1))
                y1st = wpool.tile([P, GW], F16, tag="y1st")
                nc.scalar.activation(out=y1st[:], in_=ps[:], func=COPY)
                nc.sync.dma_start(
                    out=y1b[g * GW:(g + 1) * GW, :].rearrange(
                        "(a p) f -> p a f", p=P),
                    in_=y1st[:].rearrange("p (a f) -> p a f", f=H))

            def emit_final_group(g):
                """delta rows = x @ voW.T + vo_b for vert group g."""
                ps = pp.tile([P, GW], F32, tag="bank")
                for t in range(GP):
                    col = (g * GP + t) * P
                    sl = ps[:, t * 3:(t + 1) * 3]
                    nc.tensor.matmul(out=sl, lhsT=xT[:, col:col + P],
                                     rhs=voWT_t[:, :3], start=(t == 0), stop=False)
                    nc.tensor.matmul(out=sl, lhsT=ones_t[:1, :], rhs=vob_t[:1, :3],
                                     start=False, stop=(t == GP - 1))
                nc.vector.tensor_copy(out=outstage[:, g * GP * 3:(g + 1) * GP * 3],
                                      in_=ps[:, :GP * 3])

            # ---- stage 0: xT = relu(bnW @ imgT + bnb), 512 verts per group,
            # layer-0 y1 interleaved per group
            for g in range(NG):
                ps = pp.tile([P, GW], F32, tag="bank")
                for kc in range(IMGC):
                    imgbuf = wpool.tile([P, GW], F16, tag="img")
                    nc.sync.dma_start(
                        out=imgbuf[:],
                        in_=img_T[kc * P:(kc + 1) * P, g * GW:(g + 1) * GW])
                    nc.tensor.matmul(out=ps[:], lhsT=bnWT_t[:, kc * H:(kc + 1) * H],
                                     rhs=imgbuf[:],
                                     start=(kc == 0), stop=(kc == IMGC - 1))
                nc.scalar.activation(out=xT[:, g * GW:(g + 1) * GW], in_=ps[:],
                                     func=RELU, bias=bnb_t[:, :1])
                emit_y1_group(0, g)

            # ---- layers: AllGather -> phase C (with next-layer y1 or final
            # projection interleaved per completed group)
            for l in range(NL):
                w0l = w0_t[:, l * H:(l + 1) * H]
                b0l = b0_t[:, l:l + 1]

                if "ag" not in skip:
                    nc.gpsimd.collective_compute(
                        "AllGather", mybir.AluOpType.bypass,
                        replica_groups=[list(range(NC))],
                        ins=[y1b[:, :]], outs=[y1full[:, :]])

                # phase C: scatter-reduce + y0T + relu(…+b0)
                for sg in range(NSG):
                    t0 = sg * SG
                    ntb = min(SG, TILES - t0)
                    ch0 = NC * sg * SG
                    if l == 0:
                        vbufC = wpool.tile([3, SG * P], F16, tag="vC")
                        nc.sync.dma_start(out=vbufC[:, :ntb * P],
                                          in_=vertsT[:3, t0 * P:(t0 + ntb) * P])
                    nbank = (ntb + GP - 1) // GP
                    noscat = "scatter" in skip
                    banks = []
                    for b in range(nbank):
                        ps = pp.tile([P, GW], F32, tag="bank")
                        cols = (t0 + b * GP) * P
                        nc.tensor.matmul(out=ps[:], lhsT=w0l,
                                         rhs=xT[:, cols:cols + GW],
                                         start=True, stop=(noscat and l != 0))
                        if l == 0:
                            nc.tensor.matmul(
                                out=ps[:], lhsT=w0v_t[:3, :],
                                rhs=vbufC[:3, b * GW:(b + 1) * GW],
                                start=False, stop=noscat)
                        banks.append(ps)
                    for o in range(NC):
                        if "gather" not in skip:
                            gb = gpool.tile([P, SG, H], F16, tag="g")
                            ids = idx_t[:, (ch0 + o * ntb) * 8:(ch0 + (o + 1) * ntb) * 8]
                            nc.gpsimd.dma_gather(
                                out_ap=gb[:, :ntb, :],
                                in_ap=y1full[o * VLP:(o + 1) * VLP, :],
                                idxs_ap=ids, num_idxs=ntb * P, num_idxs_reg=ntb * P,
                                elem_size=H, single_packet=False)
                        S = spool.tile([P, SG * P], F16, tag="S")
                        dr = drel_t[:, ch0 + o * ntb: ch0 + (o + 1) * ntb]
                        nc.vector.tensor_tensor(
                            out=S[:, :ntb * P].rearrange("p (c f) -> p c f", f=P),
                            in0=dr[:, :, None].to_broadcast([P, ntb, P]),
                            in1=iota_t[:, None, :].to_broadcast([P, ntb, P]),
                            op=EQ)
                        if noscat:
                            continue
                        for t in range(ntb):
                            b = t // GP
                            nc.tensor.matmul(
                                out=banks[b][:, (t % GP) * H:(t % GP + 1) * H],
                                lhsT=gb[:, t, :], rhs=S[:, t * P:(t + 1) * P],
                                start=False,
                                stop=(o == NC - 1 and (t % GP == GP - 1
                                                       or t == ntb - 1)))
                    for b in range(nbank):
                        cols = (t0 + b * GP) * P
                        nc.scalar.activation(out=xT[:, cols:cols + GW],
                                             in_=banks[b][:], func=RELU, bias=b0l)
                        g = t0 // GP + b
                        if l + 1 < NL:
                            emit_y1_group(l + 1, g)
                        else:
                            emit_final_group(g)

            nc.sync.dma_start(
                out=delta.ap().rearrange("(t p) f -> p t f", p=P),
                in_=outstage[:].rearrange("p (t f) -> p t f", f=3))

    nc.finalize()
    return nc


# ------------------------------ runner ------------------------------
import time
import numpy as np
import jax
from jax.sharding import Mesh, PartitionSpec
from jax.experimental.shard_map import shard_map
import concourse.mybir as mybir
from concourse import bass2jax
from concourse.bass2jax import _bass_exec_p, partition_id_tensor, install_neuronx_cc_hook


class SpmdRunner:
    def __init__(self, nc, n_cores=8):
        install_neuronx_cc_hook()
        self.nc = nc
        self.n_cores = n_cores
        partition_name = nc.partition_id_tensor.name if nc.partition_id_tensor else None
        in_names, out_names, out_avals, zero_outs = [], [], [], []
        for alloc in nc.m.functions[0].allocations:
            if not isinstance(alloc, mybir.MemoryLocationSet):
                continue
            name = alloc.memorylocations[0].name
            if alloc.kind == "ExternalInput":
                if name != partition_name:
                    in_names.append(name)
            elif alloc.kind == "ExternalOutput":
                out_names.append(name)
                shape = tuple(alloc.tensor_shape)
                dtype = mybir.dt.np(alloc.dtype)
                out_avals.append(jax.core.ShapedArray(shape, dtype))
                zero_outs.append(np.zeros(shape, dtype))
        self.n_params = len(in_names)
        self.out_names = list(out_names)
        self.out_avals = out_avals
        self.zero_outs = zero_outs
        all_in = in_names + out_names
        if partition_name is not None:
            all_in.append(partition_name)
        self.in_names = all_in
        n_outs = len(out_avals)
        donate = tuple(range(self.n_params, self.n_params + n_outs))

        def _body(*args):
            operands = list(args)
            if partition_name is not None:
                operands.append(partition_id_tensor())
            return tuple(_bass_exec_p.bind(
                *operands,
                out_avals=tuple(out_avals),
                in_names=tuple(self.in_names),
                out_names=tuple(out_names),
                lowering_input_output_aliases=(),
                sim_require_finite=True,
                sim_require_nnan=True,
                nc=nc,
            ))

        devices = jax.devices()[:n_cores]
        mesh = Mesh(np.asarray(devices), ("core",))
        in_specs = (PartitionSpec("core"),) * (self.n_params + n_outs)
        out_specs = (PartitionSpec("core"),) * n_outs
        self.jitted = jax.jit(
            shard_map(_body, mesh=mesh, in_specs=in_specs, out_specs=out_specs,
                      check_rep=False),
            donate_argnums=donate,
            keep_unused=True,
        )

    def _concat_inputs(self, in_maps):
        if self.nc.dbg_addr is not None:
            z = np.zeros((1, 2), np.uint32)
            in_maps = [{**m, self.nc.dbg_addr.name: z} for m in in_maps]
        per_core = [[np.asarray(m[name]) for name in self.in_names[:self.n_params]]
                    for m in in_maps]
        concat_in = [np.concatenate([per_core[c][i] for c in range(self.n_cores)], axis=0)
                     for i in range(self.n_params)]
        concat_zeros = [np.zeros((self.n_cores * z.shape[0], *z.shape[1:]), z.dtype)
                        for z in self.zero_outs]
        return concat_in, concat_zeros

    def run(self, in_maps, iters=0):
        """Returns (results_per_core, best_seconds_per_iter or None)."""
        concat_in, concat_zeros = self._concat_inputs(in_maps)
        # device_put once so timing excludes H2D
        concat_in = [jax.device_put(a) for a in concat_in]
        out_arrs = self.jitted(*concat_in, *[jax.device_put(z) for z in concat_zeros])
        jax.block_until_ready(out_arrs)
        best = None
        for _ in range(iters):
            zs = [jax.device_put(z) for z in concat_zeros]
            jax.block_until_ready(zs)
            t0 = time.perf_counter()
            out_arrs2 = self.jitted(*concat_in, *zs)
            jax.block_until_ready(out_arrs2)
            dt = time.perf_counter() - t0
            best = dt if best is None else min(best, dt)
        results = [
            {name: np.asarray(out_arrs[i]).reshape(self.n_cores, *self.out_avals[i].shape)[c]
             for i, name in enumerate(self.out_names)}
            for c in range(self.n_cores)
        ]
        return results, best


# ------------------------------ host side ------------------------------
import numpy as np


def make_in_maps(inputs, cfg):
    NC, VL, VLP, IMG, IMGC, P = (cfg["NC"], cfg["VL"], cfg["VLP"], cfg["IMG"],
                                 cfg["IMGC"], cfg["P"])
    edges = np.asarray(inputs["edges"]).astype(np.int64)
    img16 = np.asarray(inputs["img_feats"]).astype(np.float16)
    verts16 = np.asarray(inputs["verts"]).astype(np.float16)
    cores, _ = prep_all(edges, cfg)
    shared = pack_weights(inputs, cfg)
    in_maps = []
    for c in range(NC):
        d = cores[c]
        row_of = d["row_of"]
        img_T = np.zeros((IMGC * P, VLP), np.float16)
        img_T[:IMG, row_of] = img16[c * VL:(c + 1) * VL].T
        vT = np.zeros((3, VLP), np.float16)
        vT[:, row_of] = verts16[c * VL:(c + 1) * VL].T
        m = dict(shared)
        m.update(img_T=img_T, vertsT=vT, idx16=d["idx16"], dst_rel=d["dst_rel"])
        in_maps.append(m)
    return in_maps, cores


def assemble(results, cores, cfg):
    NC, VL, V = cfg["NC"], cfg["VL"], cfg["V"]
    out = np.empty((V, 3), np.float32)
    for c in range(NC):
        out[c * VL:(c + 1) * VL] = results[c]["delta"][cores[c]["row_of"]]
    return out


_CACHE = {}


def _get_runner(cfg):
    key = (cfg["V"], cfg["NL"], cfg["SG"])
    if key not in _CACHE:
        nc = build_nc(cfg)
        _CACHE[key] = SpmdRunner(nc)
    return _CACHE[key]


def kernel(**inputs):
    cfg = make_cfg()
    in_maps, cores = make_in_maps(inputs, cfg)
    r = _get_runner(cfg)
    results, _ = r.run(in_maps, iters=0)
    return assemble(results, cores, cfg)


# revision 12
# speedup vs baseline: 7.0577x; 1.0263x over previous
"""Trainium2 kernel for nn_DeformationNetworkGraphConvolutionalLite.

Self-contained: accepts FULL inputs (as in reference.setup_inputs()),
shards across 8 NeuronCores internally, returns the FULL [200000, 3] output.

Distributed GraphConv deformation network (8 NeuronCores), fp16 fast path.

Design:
  - Vertices sharded 8-way (vertex-parallel). Core c owns a contiguous range.
  - Activations live in SBUF feature-transposed: xT [128=h, VLP=v] fp16.
    All dense matmuls then run at moving-free-dim 512 with no transposes:
      y1 rows  : out[v,h'] = sum_h xT[h,v-tile] * w1[h,h']   (lhsT=xT tile)
      y0T cols : out[h',v] = sum_h w0[h,h'] * xT[h,v]        (rhs =xT cols)
    b0 + ReLU fuse into the Activation-engine PSUM->SBUF copy (per-partition
    bias); b1 is added via a 1-row ones matmul (pre-scatter, as in pytorch3d).
  - Per-core local dest relabeling (host) packs dests into 196 tiles of 128
    such that each (tile, src-owner) block holds at most 128 directed edges.
  - Per layer: y1 (fp16) -> DRAM bounce; AllGather into an 8*VLP-row fp16
    table; dma_gather (int16 idx per owner slice) pulls neighbor rows in
    [slot, h] layout; matmul with a one-hot [slot, dpos] (is_equal vs iota)
    scatter-reduces each 128-edge chunk into the dest tile's PSUM column
    block, accumulating on top of y0T.
  - Padding slots gather row 0 and carry dst_rel=-1 so the one-hot kills
    them. Everything 2-byte (fp16): half the HBM/collective/gather traffic
    and 1 PE cycle/row instead of 4.
"""
import numpy as np
import concourse.bass as bass
import concourse.bacc as bacc
import concourse.mybir as mybir
import concourse.tile as tile

F32 = mybir.dt.float32
F16 = mybir.dt.float16
I16 = mybir.dt.int16


def make_cfg(V=200000, E=600000, IMG=960, H=128, NL=8, NC=8, SG=16):
    P = 128
    VL = V // NC
    TILES = (VL + P - 1) // P
    VLP = TILES * P
    cfg = dict(V=V, E=E, IMG=IMG, H=H, NL=NL, NC=NC, P=P, VL=VL, TILES=TILES,
               VLP=VLP, TBL=NC * VLP, SG=SG,
               NSG=(TILES + SG - 1) // SG,
               NCH=NC * TILES,            # chunks per core per layer
               IMGC=(IMG + P - 1) // P,   # padded K chunks for bottleneck
               GP=4)                      # tiles per PSUM bank group
    return cfg


# ---------------------------------------------------------------- host prep

def assign_tiles(cnt, cfg):
    """Greedy balanced assignment of VL dests into TILES tiles of <=128,
    such that per-(tile, owner) edge counts stay <= 128.
    cnt: [VL, NC] per-dest per-owner in-edge counts.
    Returns row_of [VL] -> padded row index in [0, VLP)."""
    P, TILES, NC = cfg["P"], cfg["TILES"], cfg["NC"]
    VL = cfg["VL"]
    rem = np.full((TILES, NC), P, dtype=np.int64)
    slots = np.full(TILES, P, dtype=np.int64)
    order = np.argsort(-cnt.sum(1), kind="stable")
    tile_of = np.empty(VL, dtype=np.int64)
    pos_of = np.empty(VL, dtype=np.int64)
    fill = np.zeros(TILES, dtype=np.int64)
    for d in order:
        v = cnt[d]
        slack = rem - v[None, :]
        ok = (slots > 0) & (slack.min(axis=1) >= 0)
        if not ok.any():
            raise RuntimeError("tile assignment infeasible")
        score = np.where(ok, slack.min(axis=1) * 1000 + slots, -1)
        t = int(np.argmax(score))
        tile_of[d] = t
        pos_of[d] = fill[t]
        fill[t] += 1
        slots[t] -= 1
        rem[t] -= v
    return tile_of * P + pos_of, tile_of, pos_of


def prep_all(edges, cfg):
    """edges: [E,2] int64 global undirected. Returns per-core prep dicts."""
    NC, VL, P, TILES, SG, NSG = (cfg["NC"], cfg["VL"], cfg["P"], cfg["TILES"],
                                 cfg["SG"], cfg["NSG"])
    i, j = edges[:, 0], edges[:, 1]
    dd = np.concatenate([i, j])
    ss = np.concatenate([j, i])
    owner_d = dd // VL
    cores = []
    # pass 1: per-core relabeling
    for c in range(NC):
        m = owner_d == c
        dst = dd[m] - c * VL
        src = ss[m]
        so = src // VL
        cnt = np.bincount(dst * NC + so, minlength=VL * NC).reshape(VL, NC)
        row_of, tile_of, pos_of = assign_tiles(cnt, cfg)
        cores.append(dict(dst=dst, src=src, so=so, row_of=row_of,
                          tile_of=tile_of, pos_of=pos_of))
    row_of_all = [cores[c]["row_of"] for c in range(NC)]
    # block flat layout: for sg: for o: for tile-in-sg: 128 slots
    block_start = np.zeros((TILES, NC), dtype=np.int64)
    cursor = 0
    chunk_of = np.zeros((TILES, NC), dtype=np.int64)
    ch = 0
    for sg in range(NSG):
        t0, t1 = sg * SG, min((sg + 1) * SG, TILES)
        for o in range(NC):
            for t in range(t0, t1):
                block_start[t, o] = cursor
                chunk_of[t, o] = ch
                cursor += P
                ch += 1
    TOT = cursor
    assert ch == cfg["NCH"] and TOT == cfg["NCH"] * P
    # pass 2: slot arrays
    for c in range(NC):
        d = cores[c]
        t_of = d["tile_of"][d["dst"]]
        key = t_of * NC + d["so"]
        ordk = np.argsort(key, kind="stable")
        ks = key[ordk]
        starts = np.searchsorted(ks, np.arange(TILES * NC))
        pos_in_block = np.arange(len(ks)) - starts[ks]
        assert pos_in_block.max(initial=0) < P
        flat = (block_start[t_of, d["so"]][ordk] + pos_in_block)
        idx_arr = np.zeros(TOT, dtype=np.int16)
        drel = np.full(TOT, -1.0, dtype=np.float16)
        srcs = d["src"][ordk]
        local_rows = np.concatenate([row_of_all[o][None] for o in range(NC)])
        # table-local row of each src within its owner's block
        src_local_row = local_rows[srcs // VL, srcs % VL]
        assert src_local_row.max(initial=0) < cfg["VLP"] <= 32768
        idx_arr[flat] = src_local_row.astype(np.int16)
        drel[flat] = d["pos_of"][d["dst"]][ordk].astype(np.float16)
        d["idx16"] = np.ascontiguousarray(idx_arr.reshape(TOT // 16, 16).T)
        d["dst_rel"] = np.ascontiguousarray(
            drel.reshape(cfg["NCH"], P).T)
    return cores, chunk_of


def pack_weights(inp, cfg):
    """Shared (core-independent) aux weight arrays, already transposed."""
    IMG, H, NL, P, IMGC = cfg["IMG"], cfg["H"], cfg["NL"], cfg["P"], cfg["IMGC"]
    f16, f32 = np.float16, np.float32
    bnWT = np.zeros((IMGC * P, H), f16)
    bnWT[:IMG] = np.asarray(inp["bn_W"], f32).T.astype(f16)   # [IMG->pad, H]
    w0 = np.zeros((NL, H, H), f16)
    w1 = np.zeros((NL, H, H), f16)
    b0 = np.zeros((H, NL), f32)
    b1 = np.zeros((NL, H), f16)
    g0W0 = np.asarray(inp["g0_W0"], f32)                 # [H, H+3]
    g0W1 = np.asarray(inp["g0_W1"], f32)
    w0[0] = g0W0[:, :H].T.astype(f16)
    w1[0] = g0W1[:, :H].T.astype(f16)
    b0[:, 0] = np.asarray(inp["g0_b0"], f32)
    b1[0] = np.asarray(inp["g0_b1"], f16)
    for l in range(1, NL):
        w0[l] = np.asarray(inp["gW0"], f32)[l - 1].T.astype(f16)
        w1[l] = np.asarray(inp["gW1"], f32)[l - 1].T.astype(f16)
        b0[:, l] = np.asarray(inp["gb0"], f32)[l - 1]
        b1[l] = np.asarray(inp["gb1"], f32)[l - 1].astype(f16)
    w0v = np.ascontiguousarray(g0W0[:, H:].T.astype(f16))    # [3, H]
    w1v = np.ascontiguousarray(g0W1[:, H:].T.astype(f16))
    voWT = np.ascontiguousarray(np.asarray(inp["vo_W"], f32).T.astype(f16))
    return dict(bnWT=bnWT,
                bnb=np.asarray(inp["bn_b"], f32).reshape(H, 1),
                w0=w0, w1=w1, b0col=b0, b1=b1.reshape(1, -1),
                w0v=w0v, w1v=w1v,
                voWT=voWT, vob=np.asarray(inp["vo_b"], f16)[None, :],
                iota=np.tile(np.arange(P, dtype=f16), (P, 1)))


# ---------------------------------------------------------------- builder

def build_nc(cfg, skip=frozenset()):
    """skip: subset of {"ag","gather","scatter","phasea"} for ablation
    benches only — results are wrong when non-empty."""
    skip = frozenset(skip)
    P, H, NL, NC = cfg["P"], cfg["H"], cfg["NL"], cfg["NC"]
    TILES, VLP, TBL, SG, NSG, NCH = (cfg["TILES"], cfg["VLP"], cfg["TBL"],
                                     cfg["SG"], cfg["NSG"], cfg["NCH"])
    IMGC, GP = cfg["IMGC"], cfg["GP"]
    NG = TILES // GP            # 512-col groups (49)
    GW = GP * P                 # group width in verts (512)
    TOT = NCH * P
    assert TILES % GP == 0
    nc = bacc.Bacc(None, target_bir_lowering=False, debug=False)
    dp16 = lambda n, s: nc.declare_dram_parameter(n, s, F16, isOutput=False)
    img_T = dp16("img_T", [IMGC * P, VLP])
    vertsT = dp16("vertsT", [3, VLP])
    idx16 = nc.declare_dram_parameter("idx16", [16, TOT // 16], I16, isOutput=False)
    dst_rel = dp16("dst_rel", [P, NCH])
    bnWT = dp16("bnWT", [IMGC * P, H])
    bnb = nc.declare_dram_parameter("bnb", [H, 1], F32, isOutput=False)
    w0 = dp16("w0", [NL, H, H])
    w1 = dp16("w1", [NL, H, H])
    b0col = nc.declare_dram_parameter("b0col", [H, NL], F32, isOutput=False)
    b1 = dp16("b1", [1, NL * H])
    w0v = dp16("w0v", [3, H])
    w1v = dp16("w1v", [3, H])
    voWT = dp16("voWT", [H, 3])
    vob = dp16("vob", [1, 3])
    iota_in = dp16("iota", [P, P])
    delta = nc.declare_dram_parameter("delta", [VLP, 3], F32, isOutput=True)

    y1b = nc.dram_tensor("y1b", [VLP, H], F16)
    y1full = nc.dram_tensor("y1full", [TBL, H], F16, addr_space="Shared")

    RELU = mybir.ActivationFunctionType.Relu
    COPY = mybir.ActivationFunctionType.Copy
    EQ = mybir.AluOpType.is_equal

    with tile.TileContext(nc) as tc:
        with tc.tile_pool(name="const", bufs=1) as cpool, \
             tc.tile_pool(name="work", bufs=2) as wpool, \
             tc.tile_pool(name="gpool", bufs=12) as gpool, \
             tc.tile_pool(name="spool", bufs=6) as spool, \
             tc.tile_pool(name="psum", bufs=7, space="PSUM") as pp:

            # ---- resident constants
            xT = cpool.tile([P, VLP], F16, tag="xT")
            idx_t = cpool.tile([P, TOT // 16], I16, tag="idx")
            for cc in range(8):
                nc.sync.dma_start(out=idx_t[cc * 16:(cc + 1) * 16], in_=idx16[:, :])
            drel_t = cpool.tile([P, NCH], F16, tag="drel")
            nc.sync.dma_start(out=drel_t[:], in_=dst_rel[:, :])
            iota_t = cpool.tile([P, P], F16, tag="iota")
            nc.sync.dma_start(out=iota_t[:], in_=iota_in[:, :])
            ones_t = cpool.tile([1, P], F16, tag="ones")
            nc.vector.memset(ones_t[:], 1.0)
            bnWT_t = cpool.tile([P, IMGC * H], F16, tag="bnWT")
            nc.sync.dma_start(out=bnWT_t[:].rearrange("p (k h) -> p k h", h=H),
                              in_=bnWT.ap().rearrange("(k p) h -> p k h", p=P))
            bnb_t = cpool.tile([P, 1], F32, tag="bnb")
            nc.sync.dma_start(out=bnb_t[:], in_=bnb[:, :])
            w0_t = cpool.tile([P, NL * H], F16, tag="w0")
            nc.sync.dma_start(out=w0_t[:].rearrange("p (l h) -> p l h", h=H),
                              in_=w0.ap().rearrange("l p h -> p l h"))
            w1_t = cpool.tile([P, NL * H], F16, tag="w1")
            nc.sync.dma_start(out=w1_t[:].rearrange("p (l h) -> p l h", h=H),
                              in_=w1.ap().rearrange("l p h -> p l h"))
            b0_t = cpool.tile([P, NL], F32, tag="b0")
            nc.sync.dma_start(out=b0_t[:], in_=b0col[:, :])
            b1_t = cpool.tile([1, NL * H], F16, tag="b1")
            nc.sync.dma_start(out=b1_t[:], in_=b1[:, :])
            w0v_t = cpool.tile([3, H], F16, tag="w0v")
            nc.sync.dma_start(out=w0v_t[:], in_=w0v[:, :])
            w1v_t = cpool.tile([3, H], F16, tag="w1v")
            nc.sync.dma_start(out=w1v_t[:], in_=w1v[:, :])
            voWT_t = cpool.tile([P, 3], F16, tag="voWT")
            nc.sync.dma_start(out=voWT_t[:], in_=voWT[:, :])
            vob_t = cpool.tile([1, 3], F16, tag="vob")
            nc.sync.dma_start(out=vob_t[:], in_=vob[:, :])
            outstage = cpool.tile([P, TILES * 3], F32, tag="outstage")

            def emit_y1_group(l, g):
                """y1 rows for layer l, vert group g (pre-scatter bias
                included) -> y1b. Emitted right after group g's xT update
                so it overlaps the rest of the previous phase."""
                if "phasea" in skip:
                    return
                w1l = w1_t[:, l * H:(l + 1) * H]
                b1l = b1_t[:1, l * H:(l + 1) * H]
                ps = pp.tile([P, GW], F32, tag="bank")
                if l == 0:
                    vbufA = wpool.tile([3, GW], F16, tag="vA")
                    nc.sync.dma_start(out=vbufA[:],
                                      in_=vertsT[:3, g * GW:(g + 1) * GW])
                for t in range(GP):
                    col = (g * GP + t) * P
                    sl = ps[:, t * H:(t + 1) * H]
                    nc.tensor.matmul(out=sl, lhsT=xT[:, col:col + P], rhs=w1l,
                                     start=(t == 0), stop=False)
                    if l == 0:
                        nc.tensor.matmul(out=sl, lhsT=vbufA[:3, t * P:(t + 1) * P],
                                         rhs=w1v_t[:3, :], start=False, stop=False)
                    nc.tensor.matmul(out=sl, lhsT=ones_t[:1, :], rhs=b1l,
                                     start=False, stop=(t == GP - 1))
                y1st = wpool.tile([P, GW], F16, tag="y1st")
                nc.scalar.activation(out=y1st[:], in_=ps[:], func=COPY)
                nc.sync.dma_start(
                    out=y1b[g * GW:(g + 1) * GW, :].rearrange(
                        "(a p) f -> p a f", p=P),
                    in_=y1st[:].rearrange("p (a f) -> p a f", f=H))

            def emit_final_group(g):
                """delta rows = x @ voW.T + vo_b for vert group g."""
                ps = pp.tile([P, GW], F32, tag="bank")
                for t in range(GP):
                    col = (g * GP + t) * P
                    sl = ps[:, t * 3:(t + 1) * 3]
                    nc.tensor.matmul(out=sl, lhsT=xT[:, col:col + P],
                                     rhs=voWT_t[:, :3], start=(t == 0), stop=False)
                    nc.tensor.matmul(out=sl, lhsT=ones_t[:1, :], rhs=vob_t[:1, :3],
                                     start=False, stop=(t == GP - 1))
                nc.vector.tensor_copy(out=outstage[:, g * GP * 3:(g + 1) * GP * 3],
                                      in_=ps[:, :GP * 3])

            # ---- stage 0: xT = relu(bnW @ imgT + bnb), 512 verts per group,
            # layer-0 y1 interleaved per group
            for g in range(NG):
                ps = pp.tile([P, GW], F32, tag="bank")
                for kc in range(IMGC):
                    imgbuf = wpool.tile([P, GW], F16, tag="img")
                    nc.sync.dma_start(
                        out=imgbuf[:],
                        in_=img_T[kc * P:(kc + 1) * P, g * GW:(g + 1) * GW])
                    nc.tensor.matmul(out=ps[:], lhsT=bnWT_t[:, kc * H:(kc + 1) * H],
                                     rhs=imgbuf[:],
                                     start=(kc == 0), stop=(kc == IMGC - 1))
                nc.scalar.activation(out=xT[:, g * GW:(g + 1) * GW], in_=ps[:],
                                     func=RELU, bias=bnb_t[:, :1])
                emit_y1_group(0, g)

            # ---- layers: AllGather -> phase C (with next-layer y1 or final
            # projection interleaved per completed group)
            for l in range(NL):
                w0l = w0_t[:, l * H:(l + 1) * H]
                b0l = b0_t[:, l:l + 1]

                if "ag" not in skip:
                    nc.gpsimd.collective_compute(
                        "AllGather", mybir.AluOpType.bypass,
                        replica_groups=[list(range(NC))],
                        ins=[y1b[:, :]], outs=[y1full[:, :]])

                # phase C: scatter-reduce + y0T + relu(…+b0)
                for sg in range(NSG):
                    t0 = sg * SG
                    ntb = min(SG, TILES - t0)
                    ch0 = NC * sg * SG
                    if l == 0:
                        vbufC = wpool.tile([3, SG * P], F16, tag="vC")
                        nc.sync.dma_start(out=vbufC[:, :ntb * P],
                                          in_=vertsT[:3, t0 * P:(t0 + ntb) * P])
                    nbank = (ntb + GP - 1) // GP
                    noscat = "scatter" in skip
                    # issue all 8 gathers first so the SWDGE queue streams
                    # through sg boundaries without waiting on PE work
                    gbs = []
                    for o in range(NC):
                        if "gather" not in skip:
                            gb = gpool.tile([P, SG, H], F16, tag="g")
                            ids = idx_t[:, (ch0 + o * ntb) * 8:(ch0 + (o + 1) * ntb) * 8]
                            nc.gpsimd.dma_gather(
                                out_ap=gb[:, :ntb, :],
                                in_ap=y1full[o * VLP:(o + 1) * VLP, :],
                                idxs_ap=ids, num_idxs=ntb * P, num_idxs_reg=ntb * P,
                                elem_size=H, single_packet=False)
                            gbs.append(gb)
                    banks = []
                    for b in range(nbank):
                        ps = pp.tile([P, GW], F32, tag="bank")
                        cols = (t0 + b * GP) * P
                        nc.tensor.matmul(out=ps[:], lhsT=w0l,
                                         rhs=xT[:, cols:cols + GW],
                                         start=True, stop=(noscat and l != 0))
                        if l == 0:
                            nc.tensor.matmul(
                                out=ps[:], lhsT=w0v_t[:3, :],
                                rhs=vbufC[:3, b * GW:(b + 1) * GW],
                                start=False, stop=noscat)
                        banks.append(ps)
                    for o in range(NC):
                        gb = gbs[o] if gbs else None
                        S = spool.tile([P, SG * P], F16, tag="S")
                        dr = drel_t[:, ch0 + o * ntb: ch0 + (o + 1) * ntb]
                        nc.vector.tensor_tensor(
                            out=S[:, :ntb * P].rearrange("p (c f) -> p c f", f=P),
                            in0=dr[:, :, None].to_broadcast([P, ntb, P]),
                            in1=iota_t[:, None, :].to_broadcast([P, ntb, P]),
                            op=EQ)
                        if noscat:
                            continue
                        for t in range(ntb):
                            b = t // GP
                            nc.tensor.matmul(
                                out=banks[b][:, (t % GP) * H:(t % GP + 1) * H],
                                lhsT=gb[:, t, :], rhs=S[:, t * P:(t + 1) * P],
                                start=False,
                                stop=(o == NC - 1 and (t % GP == GP - 1
                                                       or t == ntb - 1)))
                    for b in range(nbank):
                        cols = (t0 + b * GP) * P
                        nc.scalar.activation(out=xT[:, cols:cols + GW],
                                             in_=banks[b][:], func=RELU, bias=b0l)
                        g = t0 // GP + b
                        if l + 1 < NL:
                            emit_y1_group(l + 1, g)
                        else:
                            emit_final_group(g)

            nc.sync.dma_start(
                out=delta.ap().rearrange("(t p) f -> p t f", p=P),
                in_=outstage[:].rearrange("p (t f) -> p t f", f=3))

    nc.finalize()
    return nc


# ------------------------------ runner ------------------------------
import time
import numpy as np
import jax
from jax.sharding import Mesh, PartitionSpec
from jax.experimental.shard_map import shard_map
import concourse.mybir as mybir
from concourse import bass2jax
from concourse.bass2jax import _bass_exec_p, partition_id_tensor, install_neuronx_cc_hook


class SpmdRunner:
    def __init__(self, nc, n_cores=8):
        install_neuronx_cc_hook()
        self.nc = nc
        self.n_cores = n_cores
        partition_name = nc.partition_id_tensor.name if nc.partition_id_tensor else None
        in_names, out_names, out_avals, zero_outs = [], [], [], []
        for alloc in nc.m.functions[0].allocations:
            if not isinstance(alloc, mybir.MemoryLocationSet):
                continue
            name = alloc.memorylocations[0].name
            if alloc.kind == "ExternalInput":
                if name != partition_name:
                    in_names.append(name)
            elif alloc.kind == "ExternalOutput":
                out_names.append(name)
                shape = tuple(alloc.tensor_shape)
                dtype = mybir.dt.np(alloc.dtype)
                out_avals.append(jax.core.ShapedArray(shape, dtype))
                zero_outs.append(np.zeros(shape, dtype))
        self.n_params = len(in_names)
        self.out_names = list(out_names)
        self.out_avals = out_avals
        self.zero_outs = zero_outs
        all_in = in_names + out_names
        if partition_name is not None:
            all_in.append(partition_name)
        self.in_names = all_in
        n_outs = len(out_avals)
        donate = tuple(range(self.n_params, self.n_params + n_outs))

        def _body(*args):
            operands = list(args)
            if partition_name is not None:
                operands.append(partition_id_tensor())
            return tuple(_bass_exec_p.bind(
                *operands,
                out_avals=tuple(out_avals),
                in_names=tuple(self.in_names),
                out_names=tuple(out_names),
                lowering_input_output_aliases=(),
                sim_require_finite=True,
                sim_require_nnan=True,
                nc=nc,
            ))

        devices = jax.devices()[:n_cores]
        mesh = Mesh(np.asarray(devices), ("core",))
        in_specs = (PartitionSpec("core"),) * (self.n_params + n_outs)
        out_specs = (PartitionSpec("core"),) * n_outs
        self.jitted = jax.jit(
            shard_map(_body, mesh=mesh, in_specs=in_specs, out_specs=out_specs,
                      check_rep=False),
            donate_argnums=donate,
            keep_unused=True,
        )

    def _concat_inputs(self, in_maps):
        if self.nc.dbg_addr is not None:
            z = np.zeros((1, 2), np.uint32)
            in_maps = [{**m, self.nc.dbg_addr.name: z} for m in in_maps]
        per_core = [[np.asarray(m[name]) for name in self.in_names[:self.n_params]]
                    for m in in_maps]
        concat_in = [np.concatenate([per_core[c][i] for c in range(self.n_cores)], axis=0)
                     for i in range(self.n_params)]
        concat_zeros = [np.zeros((self.n_cores * z.shape[0], *z.shape[1:]), z.dtype)
                        for z in self.zero_outs]
        return concat_in, concat_zeros

    def run(self, in_maps, iters=0):
        """Returns (results_per_core, best_seconds_per_iter or None)."""
        concat_in, concat_zeros = self._concat_inputs(in_maps)
        # device_put once so timing excludes H2D
        concat_in = [jax.device_put(a) for a in concat_in]
        out_arrs = self.jitted(*concat_in, *[jax.device_put(z) for z in concat_zeros])
        jax.block_until_ready(out_arrs)
        best = None
        for _ in range(iters):
            zs = [jax.device_put(z) for z in concat_zeros]
            jax.block_until_ready(zs)
            t0 = time.perf_counter()
            out_arrs2 = self.jitted(*concat_in, *zs)
            jax.block_until_ready(out_arrs2)
            dt = time.perf_counter() - t0
            best = dt if best is None else min(best, dt)
        results = [
            {name: np.asarray(out_arrs[i]).reshape(self.n_cores, *self.out_avals[i].shape)[c]
             for i, name in enumerate(self.out_names)}
            for c in range(self.n_cores)
        ]
        return results, best


# ------------------------------ host side ------------------------------
import numpy as np


def make_in_maps(inputs, cfg):
    NC, VL, VLP, IMG, IMGC, P = (cfg["NC"], cfg["VL"], cfg["VLP"], cfg["IMG"],
                                 cfg["IMGC"], cfg["P"])
    edges = np.asarray(inputs["edges"]).astype(np.int64)
    img16 = np.asarray(inputs["img_feats"]).astype(np.float16)
    verts16 = np.asarray(inputs["verts"]).astype(np.float16)
    cores, _ = prep_all(edges, cfg)
    shared = pack_weights(inputs, cfg)
    in_maps = []
    for c in range(NC):
        d = cores[c]
        row_of = d["row_of"]
        img_T = np.zeros((IMGC * P, VLP), np.float16)
        img_T[:IMG, row_of] = img16[c * VL:(c + 1) * VL].T
        vT = np.zeros((3, VLP), np.float16)
        vT[:, row_of] = verts16[c * VL:(c + 1) * VL].T
        m = dict(shared)
        m.update(img_T=img_T, vertsT=vT, idx16=d["idx16"], dst_rel=d["dst_rel"])
        in_maps.append(m)
    return in_maps, cores


def assemble(results, cores, cfg):
    NC, VL, V = cfg["NC"], cfg["VL"], cfg["V"]
    out = np.empty((V, 3), np.float32)
    for c in range(NC):
        out[c * VL:(c + 1) * VL] = results[c]["delta"][cores[c]["row_of"]]
    return out


_CACHE = {}


def _get_runner(cfg):
    key = (cfg["V"], cfg["NL"], cfg["SG"])
    if key not in _CACHE:
        nc = build_nc(cfg)
        _CACHE[key] = SpmdRunner(nc)
    return _CACHE[key]


def kernel(**inputs):
    cfg = make_cfg()
    in_maps, cores = make_in_maps(inputs, cfg)
    r = _get_runner(cfg)
    results, _ = r.run(in_maps, iters=0)
    return assemble(results, cores, cfg)
